# revision 37
# baseline (speedup 1.0000x reference)
"""Trainium2 Bass kernel for MHA block (LN -> QKV -> qk-LN -> RoPE -> masked attn -> out-proj).

Self-contained: hardcodes shapes B=2, L=2048, D=1024, H=16, Dh=64; runs on 8 NeuronCores
via bass_utils.run_bass_kernel_spmd. Sharding: core c = (batch b = c//4, head-group
g = c%4 of 4 heads). Weight columns are sliced per core so "our" 4 heads are always
columns 0:256 -> the device program is identical on all cores (SPMD). The qk-LN
statistics (over the full 1024 dims) are formed from per-core partial sums with two
4-core-group AllReduces (split in halves to hide latency). RoPE is applied to the raw
q/k (it is linear) and the LN affine is folded in afterwards:
rot(LN(q)) = rstd*rot(q) - (rstd*mu)*rot(ones).

seq_id is sorted -> the attention mask is block diagonal. The host computes, per
256-query group, the key-tile range needed (union over the 2 batches so the SPMD
program stays shared) and the kernel only computes those (qgroup, key-tile) score/PV
units (~40% of dense). Attention is split into phase A (first-half queries x
first-half keys, runs while the 2nd stats AllReduce is in flight) and phase B (the
rest); partial context/denominator sums are additive, so phase A closes its partial
units into SBUF and phase B adds the remainder. Most operands are bf16 (matmul
accumulation stays fp32 in PSUM). Host sums the 4 partial out-projections per batch.
"""

import numpy as np
import ml_dtypes
from contextlib import ExitStack

import concourse.bass as bass
import concourse.tile as tile
from concourse import bacc, mybir
from concourse import bass_utils

F32 = mybir.dt.float32
BF16 = mybir.dt.bfloat16
AF = mybir.ActivationFunctionType
ALU = mybir.AluOpType

B, L, D = 2, 2048, 1024
H, DH = 16, 64
HPC = 4          # heads per core
CD = HPC * DH    # ctx dims per core = 256
P = 128
TT = L // P      # 16 token tiles
KC = D // P      # 8 contraction chunks
QG = 256         # query group width for block-sparse attention
NG = L // QG     # 8 query groups
EPS = 1e-5
ROPE_BASE = 10000.0
MASK_A = 8.0     # mask row scale; mask bias = -MASK_A^2 = -64 for masked pairs
KR = DH + 5      # contraction rows for scores (64 dims + 5 mask rows)
VB = DH + 1      # v block width (64 dims + ones col)
RG = [[0, 1, 2, 3], [4, 5, 6, 7]]


def _bcast_free(ap, n, axis):
    """Insert a step-0 free dim of size n at position `axis` (after partition dim)."""
    new = list(ap.ap)
    new.insert(axis, [0, n])
    return bass.AP(tensor=ap.tensor, offset=ap.offset, ap=new)


def _build_units(qgr):
    """Split (head, qgroup, key-tiles) into phase A (both halves finalized early)
    and phase B units. mode: 'copy' closes into craw, 'add' accumulates into it."""
    ua, ub = [], []
    for h in range(HPC):
        for g in range(NG):
            lo, hi = qgr[g]
            kts = list(range(lo, hi))
            if g < NG // 2:
                ka = [k for k in kts if k < TT // 2]
                kb = [k for k in kts if k >= TT // 2]
                if ka:
                    ua.append((h, g, ka, 'copy'))
                if kb:
                    ub.append((h, g, kb, 'add' if ka else 'copy'))
            else:
                ub.append((h, g, kts, 'copy'))
    return ua, ub


def build_bass(use_ln1b=False, use_qlw=False, use_klw=False,
               qgr=tuple((0, TT) for _ in range(NG))):
    nc = bacc.Bacc("TRN2", target_bir_lowering=False, debug=False, num_devices=8)
    use_lw = use_qlw or use_klw

    # ---- DRAM I/O ----
    x_d = nc.dram_tensor("x", [L, D], BF16, kind="ExternalInput").ap()
    wq_d = nc.dram_tensor("wq", [P, KC, CD], BF16, kind="ExternalInput").ap()
    wk_d = nc.dram_tensor("wk", [P, KC, CD], BF16, kind="ExternalInput").ap()
    wv_d = nc.dram_tensor("wv", [P, KC, CD], BF16, kind="ExternalInput").ap()
    wo_d = nc.dram_tensor("wo", [P, CD // P, D], BF16, kind="ExternalInput").ap()
    mq_d = nc.dram_tensor("maskq", [5, L], BF16, kind="ExternalInput").ap()
    mk_d = nc.dram_tensor("maskk", [5, L], BF16, kind="ExternalInput").ap()
    cos_d = nc.dram_tensor("cos", [P, TT, DH], BF16, kind="ExternalInput").ap()
    sinl_d = nc.dram_tensor("sinl", [P, TT, DH // 2], BF16, kind="ExternalInput").ap()
    sinh_d = nc.dram_tensor("sinh", [P, TT, DH // 2], BF16, kind="ExternalInput").ap()
    r1_d = nc.dram_tensor("r1", [P, TT, DH], BF16, kind="ExternalInput").ap()
    idf_d = nc.dram_tensor("identf", [P, P], F32, kind="ExternalInput").ap()
    idb_d = nc.dram_tensor("identb", [P, P], BF16, kind="ExternalInput").ap()
    if use_ln1b:
        lnb_d = nc.dram_tensor("lnb", [1, D], F32, kind="ExternalInput").ap()
    if use_qlw:
        qlw_d = nc.dram_tensor("qlw", [1, CD], F32, kind="ExternalInput").ap()
    if use_klw:
        klw_d = nc.dram_tensor("klw", [1, CD], F32, kind="ExternalInput").ap()
    out_d = nc.dram_tensor("out", [L, D], BF16, kind="ExternalOutput").ap()

    x_t_d = x_d.rearrange("(n p) d -> n p d", p=P)
    out_t_d = out_d.rearrange("(n p) d -> n p d", p=P)

    units_a, units_b = _build_units(qgr)

    with tile.TileContext(nc) as tc, ExitStack() as ctx:
        cpool = ctx.enter_context(tc.tile_pool(name="cpool", bufs=1))
        small = ctx.enter_context(tc.tile_pool(name="small", bufs=4))

        identb = cpool.tile([P, P], BF16)
        nc.sync.dma_start(identb, idb_d)
        identf = cpool.tile([P, P], F32)
        eps_ap = cpool.tile([P, 1], F32)
        nc.vector.memset(eps_ap, EPS)

        # v augmented: flat [128, TT*HPC*65 + 63]; per (kt,h) block of 65 cols
        # (64 v dims + ones col). PV reads 128 cols per block: the 63 cols past a
        # block belong to the next block -> garbage rows 65:128 in ctx psum, unread.
        pB = ctx.enter_context(tc.tile_pool(name="pB", bufs=1))
        v_sb = pB.tile([P, TT * HPC * VB + (P - VB)], BF16)
        v_blocks = v_sb[:, : TT * HPC * VB].rearrange("p (t h d) -> p t h d", t=TT, h=HPC)
        nc.gpsimd.memset(v_blocks[:, :, :, DH : DH + 1], 1.0)

        # qT/kT augmented per head: rows 0:64 = head dims (transposed), 64:69 = mask
        # rows -> scores+mask in ONE matmul over 69 contraction rows.
        qT = pB.tile([P, HPC, L], BF16)
        kT = pB.tile([P, HPC, L], BF16)
        # rope'd+LN-finalized q/k in token-major bf16, awaiting transpose
        rotb_q = pB.tile([P, TT, HPC, DH], BF16)
        rotb_k = pB.tile([P, TT, HPC, DH], BF16)
        craw_all = pB.tile([DH + 1, HPC, L], BF16)
        ctxT = pB.tile([P, CD // P, L], BF16)

        def rotb_at(j, t):
            return (rotb_q if j == 0 else rotb_k)[:, t, :, :]

        # ============ Phase 1: LN1 + QKV (our 768 cols) + partial stats + raw rope
        with ExitStack() as phA:
            pA = phA.enter_context(tc.tile_pool(name="pA", bufs=1))
            stats_pack = pA.tile([P, TT, 2, 2], F32)
            allred = pA.tile([P, TT, 2, 2], F32)
            rot_q = pA.tile([P, TT, HPC, DH], F32)
            rot_k = pA.tile([P, TT, HPC, DH], F32)

            def rot_at(j, t):
                return (rot_q if j == 0 else rot_k)[:, t, :, :]

            wq_sb = pA.tile([P, KC, CD], BF16)
            wk_sb = pA.tile([P, KC, CD], BF16)
            wv_sb = pA.tile([P, KC, CD], BF16)
            r1_sb = pA.tile([P, TT, DH], BF16)
            if use_lw:
                q4_all = pA.tile([P, TT, CD], F32)
                k4_all = pA.tile([P, TT, CD], F32)
                cos2_sb = pA.tile([P, TT, DH], BF16)
                nc.sync.dma_start(cos2_sb, cos_d)
                sinl2_sb = pA.tile([P, TT, DH // 2], BF16)
                nc.sync.dma_start(sinl2_sb, sinl_d)
                sinh2_sb = pA.tile([P, TT, DH // 2], BF16)
                nc.sync.dma_start(sinh2_sb, sinh_d)
                if use_qlw:
                    qlw_sb = pA.tile([P, CD], F32)
                    nc.sync.dma_start(qlw_sb, qlw_d.partition_broadcast(P)[:, 0, :])
                if use_klw:
                    klw_sb = pA.tile([P, CD], F32)
                    nc.sync.dma_start(klw_sb, klw_d.partition_broadcast(P)[:, 0, :])
            dramp = phA.enter_context(tc.tile_pool(name="dramp", bufs=1, space="DRAM"))
            ib1 = dramp.tile([P, TT * 2], F32)
            ob1 = dramp.tile([P, TT * 2], F32)
            ib2 = dramp.tile([P, TT * 2], F32)
            ob2 = dramp.tile([P, TT * 2], F32)

            def finalize_apply(lo, hi, after_group=None):
                """qk-LN: mu/rstd from all-reduced sums, fold into raw-rope'd q/k.
                rotb = rstd * (rot + (-mu) * r1): one DVE STT + one ACT scale per
                (tile, q/k). after_group(g4) fires once each 4-tile group is done."""
                n = hi - lo
                mu = small.tile([P, n, 2], F32, tag="fmu")
                nc.vector.tensor_scalar(mu, allred[:, lo:hi, :, 0], 1.0 / D, None, ALU.mult)
                m2 = small.tile([P, n, 2], F32, tag="fm2")
                nc.vector.tensor_mul(m2, mu, mu)
                rstd = small.tile([P, n, 2], F32, tag="frstd")
                nc.vector.scalar_tensor_tensor(
                    rstd, allred[:, lo:hi, :, 1], 1.0 / D, m2,
                    op0=ALU.mult, op1=ALU.subtract,
                )
                nc.scalar.activation(rstd, rstd, AF.Sqrt, bias=eps_ap)
                nc.vector.reciprocal(rstd, rstd)
                negmu = small.tile([P, n, 2], F32, tag="fnegmu")
                nc.vector.tensor_scalar(negmu, mu, -1.0, None, ALU.mult)
                nm = small.tile([P, n, 2], F32, tag="fnm")
                if use_lw:
                    nc.vector.scalar_tensor_tensor(nm, mu, -1.0, rstd, op0=ALU.mult, op1=ALU.mult)
                for t in range(lo, hi):
                    for j in range(2):
                        lw_flag = use_qlw if j == 0 else use_klw
                        if lw_flag:
                            src4 = q4_all if j == 0 else k4_all
                            lw_sb = qlw_sb if j == 0 else klw_sb
                            xn = small.tile([P, HPC, DH], F32, tag="xnf")
                            nc.scalar.activation(
                                xn.rearrange("p h d -> p (h d)"), src4[:, t, :],
                                AF.Identity, bias=nm[:, t - lo, j : j + 1],
                                scale=rstd[:, t - lo, j : j + 1],
                            )
                            nc.vector.tensor_mul(
                                xn, xn, lw_sb.rearrange("p (h d) -> p h d", h=HPC)
                            )
                            qa2 = small.tile([P, HPC, DH], F32, tag="qa2")
                            nc.vector.tensor_mul(
                                qa2, xn, _bcast_free(cos2_sb[:, t, :], HPC, 1)
                            )
                            qb2 = small.tile([P, HPC, DH], F32, tag="qb2")
                            nc.vector.tensor_mul(
                                qb2[:, :, 0 : DH // 2], xn[:, :, DH // 2 : DH],
                                _bcast_free(sinl2_sb[:, t, :], HPC, 1),
                            )
                            nc.vector.tensor_mul(
                                qb2[:, :, DH // 2 : DH], xn[:, :, 0 : DH // 2],
                                _bcast_free(sinh2_sb[:, t, :], HPC, 1),
                            )
                            nc.vector.tensor_add(rotb_at(j, t), qa2, qb2)
                            continue
                        tmp = small.tile([P, HPC, DH], F32, tag=f"tmp{j}", bufs=2)
                        if j == 0:
                            nc.vector.scalar_tensor_tensor(
                                tmp, _bcast_free(r1_sb[:, t, :], HPC, 1),
                                negmu[:, t - lo, j : j + 1], rot_at(j, t),
                                op0=ALU.mult, op1=ALU.add,
                            )
                        else:
                            tmp1 = small.tile([P, HPC, DH], F32, tag="tmp1g", bufs=2)
                            nc.gpsimd.tensor_mul(
                                tmp1, _bcast_free(r1_sb[:, t, :], HPC, 1),
                                _bcast_free(_bcast_free(
                                    negmu[:, t - lo, j : j + 1], HPC, 1), DH, 2)[:, :, :, 0],
                            )
                            nc.gpsimd.tensor_add(tmp, tmp1, rot_at(j, t))
                        nc.scalar.activation(
                            rotb_at(j, t).rearrange("p h d -> p (h d)"),
                            tmp.rearrange("p h d -> p (h d)"),
                            AF.Identity, scale=rstd[:, t - lo, j : j + 1],
                        )
                    if after_group is not None and t % 4 == 3:
                        after_group(t // 4)

            with ExitStack() as pctx:
                pp_qkv = pctx.enter_context(tc.tile_pool(name="pp_qkv", bufs=6, space="PSUM"))
                pp_ht = pctx.enter_context(tc.tile_pool(name="pp_ht", bufs=2, space="PSUM"))
                p1 = pctx.enter_context(tc.tile_pool(name="p1", bufs=2))

                x_pre = []
                for t in range(2):
                    x_t = p1.tile([P, D], BF16, tag="x_t", bufs=16, name=f"x_pre{t}")
                    nc.sync.dma_start(x_t, x_t_d[t])
                    x_pre.append(x_t)
                nc.sync.dma_start(wq_sb, wq_d)
                for t in range(2, 4):
                    x_t = p1.tile([P, D], BF16, tag="x_t", bufs=16, name=f"x_pre{t}")
                    nc.sync.dma_start(x_t, x_t_d[t])
                    x_pre.append(x_t)
                nc.sync.dma_start(wk_sb, wk_d)
                cos_sb = p1.tile([P, TT, DH], BF16, bufs=1)
                nc.sync.dma_start(cos_sb, cos_d)
                sinl_sb = p1.tile([P, TT, DH // 2], BF16, bufs=1)
                nc.sync.dma_start(sinl_sb, sinl_d)
                sinh_sb = p1.tile([P, TT, DH // 2], BF16, bufs=1)
                nc.sync.dma_start(sinh_sb, sinh_d)
                nc.sync.dma_start(wv_sb, wv_d)
                for t in range(4, 8):
                    x_t = p1.tile([P, D], BF16, tag="x_t", bufs=16, name=f"x_pre{t}")
                    nc.sync.dma_start(x_t, x_t_d[t])
                    x_pre.append(x_t)
                nc.sync.dma_start(r1_sb, r1_d)
                for t in range(8, TT):
                    x_t = p1.tile([P, D], BF16, tag="x_t", bufs=16, name=f"x_pre{t}")
                    nc.sync.dma_start(x_t, x_t_d[t])
                    x_pre.append(x_t)
                if use_ln1b:
                    lnb_sb = p1.tile([P, D], F32, bufs=1)
                    nc.sync.dma_start(lnb_sb, lnb_d.partition_broadcast(P)[:, 0, :])

                def stage1(t):
                    """LN1 + hT (DMA transpose) + QKV matmuls."""
                    x_t = x_pre[t]

                    xstats = small.tile([P, 2, 6], F32, tag="xstats")
                    for s in range(2):
                        nc.vector.bn_stats(
                            xstats[:, s, :],
                            x_t[:, s * 512 : (s + 1) * 512].rearrange(
                                "p (s d) -> p s d", s=1
                            ),
                        )
                    xmv = small.tile([P, 2], F32, tag="xmv")
                    nc.vector.bn_aggr(xmv, xstats)
                    xrstd = small.tile([P, 1], F32, tag="xrstd")
                    nc.scalar.activation(xrstd, xmv[:, 1:2], AF.Sqrt, bias=eps_ap)
                    nc.vector.reciprocal(xrstd, xrstd)
                    xnm = small.tile([P, 1], F32, tag="xnm")
                    nc.vector.tensor_scalar(xnm, xmv[:, 0:1], xrstd, -1.0, ALU.mult, ALU.mult)
                    h_t = p1.tile([P, D], BF16, tag="h_t", bufs=3)
                    nc.scalar.activation(h_t, x_t, AF.Identity, bias=xnm, scale=xrstd)
                    if use_ln1b:
                        nc.vector.tensor_add(h_t, h_t, lnb_sb)

                    ht_ps = pp_ht.tile([P, KC, P], BF16, tag="ht")
                    for c in range(KC):
                        nc.tensor.transpose(ht_ps[:, c, :], h_t[:, c * P : (c + 1) * P], identb)
                    hT_t = p1.tile([P, KC, P], BF16, tag="hT_t", bufs=2)
                    nc.scalar.copy(hT_t, ht_ps)

                    q_ps = pp_qkv.tile([P, CD], F32, tag="qkv", name="q_ps")
                    k_ps = pp_qkv.tile([P, CD], F32, tag="qkv", name="k_ps")
                    v_ps = pp_qkv.tile([P, CD], F32, tag="qkv", name="v_ps")
                    for c in range(KC):
                        nc.tensor.matmul(q_ps, hT_t[:, c, :], wq_sb[:, c, :],
                                         start=(c == 0), stop=(c == KC - 1))
                    for c in range(KC):
                        nc.tensor.matmul(k_ps, hT_t[:, c, :], wk_sb[:, c, :],
                                         start=(c == 0), stop=(c == KC - 1))
                    for c in range(KC):
                        nc.tensor.matmul(v_ps, hT_t[:, c, :], wv_sb[:, c, :],
                                         start=(c == 0), stop=(c == KC - 1))
                    return t, q_ps, k_ps, v_ps

                def stage2(st):
                    """Drain q/k/v psums, stats accums, raw rope."""
                    t, q_ps, k_ps, v_ps = st
                    if use_lw:
                        q4_t = q4_all[:, t, :]
                        k4_t = k4_all[:, t, :]
                    else:
                        q4_t = p1.tile([P, CD], F32, tag="q4t", bufs=3)
                        k4_t = p1.tile([P, CD], F32, tag="k4t", bufs=3)
                    nc.scalar.activation(
                        q4_t, q_ps, AF.Copy, accum_out=stats_pack[:, t, 0, 0:1]
                    )
                    nc.scalar.activation(
                        k4_t, k_ps, AF.Copy, accum_out=stats_pack[:, t, 1, 0:1]
                    )
                    nc.scalar.copy(
                        v_blocks[:, t, :, 0:DH],
                        v_ps.rearrange("p (h d) -> p h d", h=HPC),
                    )
                    # s2 = rowsum(q4^2) on DVE, from SBUF
                    sq = p1.tile([P, CD], F32, tag="sq", bufs=2)
                    nc.vector.scalar_tensor_tensor(
                        sq, q4_t, 1.0, q4_t, op0=ALU.mult, op1=ALU.mult,
                        accum_out=stats_pack[:, t, 0, 1:2],
                    )
                    nc.vector.scalar_tensor_tensor(
                        sq, k4_t, 1.0, k4_t, op0=ALU.mult, op1=ALU.mult,
                        accum_out=stats_pack[:, t, 1, 1:2],
                    )

                    # raw rope (linear; LN affine folded in afterwards), from SBUF
                    for j in range(2):
                        src4 = q4_t if j == 0 else k4_t
                        xn4 = src4.rearrange("p (h d) -> p h d", h=HPC)
                        qa = p1.tile([P, HPC, DH], F32, tag="qa", bufs=2)
                        nc.gpsimd.tensor_mul(qa, xn4, _bcast_free(cos_sb[:, t, :], HPC, 1))
                        qb = p1.tile([P, HPC, DH], F32, tag="qb", bufs=2)
                        nc.gpsimd.tensor_mul(
                            qb[:, :, 0 : DH // 2],
                            xn4[:, :, DH // 2 : DH],
                            _bcast_free(sinl_sb[:, t, :], HPC, 1),
                        )
                        nc.gpsimd.tensor_mul(
                            qb[:, :, DH // 2 : DH],
                            xn4[:, :, 0 : DH // 2],
                            _bcast_free(sinh_sb[:, t, :], HPC, 1),
                        )
                        nc.vector.tensor_add(rot_at(j, t), qa, qb)

                # two-stage software pipeline: stage1(t) runs one tile ahead of
                # stage2(t-1) so PE never waits on the psum-drain/rope tail.
                st_prev = None
                for t in range(TT):
                    st = stage1(t)
                    if st_prev is not None:
                        stage2(st_prev)
                        if t == 8:
                            # first-half AllReduce; hides under tiles 8-15
                            nc.gpsimd.dma_start(
                                ib1[:],
                                stats_pack[:, 0:8, :, :].rearrange("p t j s -> p (t j s)"),
                            )
                            nc.gpsimd.collective_compute(
                                "AllReduce", ALU.add, replica_groups=RG,
                                ins=[ib1.opt()], outs=[ob1.opt()],
                            )
                            nc.sync.dma_start(
                                allred[:, 0:8, :, :].rearrange("p t j s -> p (t j s)"),
                                ob1[:],
                            )
                    st_prev = st
                stage2(st_prev)

            # late constants: mask rows + fp32 identity (not needed until here)
            for hh in range(HPC):
                nc.sync.dma_start(qT[DH : DH + 5, hh, :], mq_d)
                nc.sync.dma_start(kT[DH : DH + 5, hh, :], mk_d)
            nc.sync.dma_start(identf, idf_d)

            # second-half AllReduce launched immediately; its latency is hidden
            # under finalize(0:8) + first-half transposes + phase-A attention.
            nc.gpsimd.dma_start(
                ib2[:],
                stats_pack[:, 8:16, :, :].rearrange("p t j s -> p (t j s)"),
            )
            nc.gpsimd.collective_compute(
                "AllReduce", ALU.add, replica_groups=RG,
                ins=[ib2.opt()], outs=[ob2.opt()],
            )
            nc.sync.dma_start(
                allred[:, 8:16, :, :].rearrange("p t j s -> p (t j s)"),
                ob2[:],
            )

            # ============ Phase 2: attention (A while AR2 in flight, then B)
            with ExitStack() as actx:
                pa_sc = actx.enter_context(tc.tile_pool(name="pa_sc", bufs=3, space="PSUM"))
                pa_ctx = actx.enter_context(tc.tile_pool(name="pa_ctx", bufs=3, space="PSUM"))
                pp_tr = actx.enter_context(tc.tile_pool(name="pp_tr", bufs=2, space="PSUM"))
                p2 = actx.enter_context(tc.tile_pool(name="p2", bufs=4))

                def emit_transposes_group(g4):
                    for j in range(2):
                        dst = qT if j == 0 else kT
                        for hh in range(HPC):
                            tp = pp_tr.tile([DH, 4, P], BF16, tag="tp",
                                            name=f"tp{j}{hh}{g4}")
                            for i in range(4):
                                nc.tensor.transpose(
                                    tp[:, i, :], rotb_at(j, g4 * 4 + i)[:, hh, :],
                                    identb,
                                )
                            if hh % 2 == 0:
                                nc.scalar.copy(
                                    dst[0:DH, hh, g4 * 512 : (g4 + 1) * 512],
                                    tp.rearrange("p g t -> p (g t)"),
                                )
                            else:
                                nc.vector.tensor_copy(
                                    dst[0:DH, hh, g4 * 512 : (g4 + 1) * 512],
                                    tp.rearrange("p g t -> p (g t)"),
                                )

                def emit_attn(units):
                    # flatten to chunks of <=2 key tiles; 2-chunk software pipeline
                    chunks = []
                    for ui, (h, g, kts, mode) in enumerate(units):
                        for ci in range(0, len(kts), 2):
                            sub = kts[ci : ci + 2]
                            chunks.append({
                                "h": h, "g": g, "kts": sub, "mode": mode, "ui": ui,
                                "first": ci == 0, "last": ci + 2 >= len(kts),
                            })
                    cur_ctx = [None]

                    def emit_pv(c):
                        nk = len(c["kts"])
                        for i, kt in enumerate(c["kts"]):
                            voff = (kt * HPC + c["h"]) * VB
                            nc.tensor.matmul(
                                c["ctx"], v_sb[:, voff : voff + P],
                                c["eT"][:, i * QG : (i + 1) * QG],
                                start=(c["first"] and i == 0),
                                stop=(c["last"] and i == nk - 1),
                            )
                        if c["last"]:
                            dst = craw_all[:, c["h"], c["g"] * QG : (c["g"] + 1) * QG]
                            if c["mode"] == "add":
                                nc.vector.tensor_add(dst, dst, c["ctx"][0 : DH + 1, :])
                            else:
                                nc.vector.tensor_copy(dst, c["ctx"][0 : DH + 1, :])

                    pend = []
                    for c in chunks:
                        nk = len(c["kts"])
                        s_ps = pa_sc.tile([P, 2 * QG], F32, tag="sc", name="s_ps")
                        for i, kt in enumerate(c["kts"]):
                            nc.tensor.matmul(
                                s_ps[:, i * QG : (i + 1) * QG],
                                kT[0:KR, c["h"], kt * P : (kt + 1) * P],
                                qT[0:KR, c["h"], c["g"] * QG : (c["g"] + 1) * QG],
                                start=True, stop=True,
                            )
                        eT = p2.tile([P, 2 * QG], BF16, tag="eT", name="eT")
                        nc.scalar.activation(
                            eT[:, : nk * QG], s_ps[:, : nk * QG], AF.Exp
                        )
                        c["eT"] = eT
                        if c["first"]:
                            cur_ctx[0] = pa_ctx.tile([P, QG], F32, tag="ctx",
                                                     name="ctx_ps", bufs=3)
                        c["ctx"] = cur_ctx[0]
                        pend.append(c)
                        if len(pend) > 2:
                            emit_pv(pend.pop(0))
                    for c in pend:
                        emit_pv(c)

                wo_sb = p2.tile([P, CD // P, D], BF16, tag="wo", bufs=1)
                nc.sync.dma_start(wo_sb, wo_d)

                def emit_outproj_grp(grp):
                    """Normalize craw for qgroups 2*grp..2*grp+1 and project out."""
                    for h in range(HPC):
                        pr, sub = h // 2, h % 2
                        rows = slice(sub * DH, (sub + 1) * DH)
                        fwd = pp_tr.tile([P, 4, DH + 2], BF16, tag="tp",
                                         name=f"fwd{grp}{h}")
                        for i in range(4):
                            tix = grp * 4 + i
                            nc.tensor.transpose(
                                fwd[:, i, 0 : DH + 1],
                                craw_all[:, h, tix * P : (tix + 1) * P],
                                identb[: DH + 1, : DH + 1],
                            )
                        rz = small.tile([P, 4], F32, tag="rz")
                        nc.vector.reciprocal(rz, fwd[:, :, DH])
                        cn = p2.tile([P, 4, DH], BF16, tag="cn", bufs=2,
                                     name=f"cn{grp}{h}")
                        nc.vector.tensor_mul(cn, fwd[:, :, 0:DH], _bcast_free(rz, DH, 2))
                        back = pp_tr.tile([DH, 4, P], BF16, tag="tp",
                                          name=f"back{grp}{h}")
                        for i in range(4):
                            nc.tensor.transpose(back[:, i, :], cn[:, i, :], identb)
                        nc.vector.tensor_copy(
                            ctxT[rows, pr, grp * 512 : (grp + 1) * 512],
                            back.rearrange("p g t -> p (g t)"),
                        )
                    for i in range(4):
                        t = grp * 4 + i
                        o_ps = [pa_ctx.tile([P, 512], F32, tag="ctx", bufs=3,
                                            name=f"o_ps{t}{s}")
                                for s in range(2)]
                        for s in range(2):
                            for c in range(CD // P):
                                nc.tensor.matmul(
                                    o_ps[s], ctxT[:, c, t * P : (t + 1) * P],
                                    wo_sb[:, c, s * 512 : (s + 1) * 512],
                                    start=(c == 0), stop=(c == CD // P - 1),
                                )
                        o_t = p2.tile([P, D], BF16, tag="o_t", bufs=2, name=f"o_t{t}")
                        nc.scalar.copy(o_t[:, 0:512], o_ps[0])
                        nc.vector.tensor_copy(o_t[:, 512:1024], o_ps[1])
                        nc.sync.dma_start(out_t_d[t], o_t)

                finalize_apply(0, 8, after_group=emit_transposes_group)
                emit_attn(units_a)
                finalize_apply(8, 16, after_group=emit_transposes_group)
                # phase B grouped by qgroup pairs so each outproj group starts
                # as soon as its craw columns close
                ub = sorted(units_b, key=lambda u: (u[1], u[0]))
                for grp in range(4):
                    gs = (2 * grp, 2 * grp + 1)
                    emit_attn([u for u in ub if u[1] in gs])
                    emit_outproj_grp(grp)

    nc.compile()
    return nc


_CACHE = {}


def _get_nc(key):
    if key not in _CACHE:
        _CACHE[key] = build_bass(*key)
    return _CACHE[key]


def _qg_ranges(seq_id):
    """Per 256-query group: key-tile range (lo, hi) needed, unioned over batches.
    Falls back to dense if any row is unsorted."""
    for b in range(seq_id.shape[0]):
        if np.any(np.diff(seq_id[b].astype(np.int64)) < 0):
            return tuple((0, TT) for _ in range(NG))
    rs = []
    for g in range(NG):
        lo, hi = L, 0
        for b in range(seq_id.shape[0]):
            s = seq_id[b]
            l = int(np.searchsorted(s, s[g * QG], 'left'))
            h_ = int(np.searchsorted(s, s[(g + 1) * QG - 1], 'right'))
            lo = min(lo, l)
            hi = max(hi, h_)
        rs.append((lo // P, -(-hi // P)))
    return tuple(rs)


def _host_prep(x, seq_id, ln1_w, ln1_b, w_qkv, q_ln_w, k_ln_w, w_out):
    """Build the 8 per-core input maps."""
    x = np.asarray(x, np.float32)
    seq_id = np.asarray(seq_id)
    ln1_w = np.asarray(ln1_w, np.float32)
    ln1_b = np.asarray(ln1_b, np.float32)
    w_qkv = np.asarray(w_qkv, np.float32)
    q_ln_w = np.asarray(q_ln_w, np.float32)
    k_ln_w = np.asarray(k_ln_w, np.float32)
    w_out = np.asarray(w_out, np.float32)

    use_ln1b = bool(np.any(ln1_b != 0.0))
    use_qlw = not np.allclose(q_ln_w, 1.0)
    use_klw = not np.allclose(k_ln_w, 1.0)
    qgr = _qg_ranges(seq_id)

    BD = ml_dtypes.bfloat16
    wq_f = (w_qkv[:, 0:D] * ln1_w[:, None]).astype(BD)
    wk_f = (w_qkv[:, D : 2 * D] * ln1_w[:, None]).astype(BD)
    wv_f = (w_qkv[:, 2 * D : 3 * D] * ln1_w[:, None]).astype(BD)
    wo_b = w_out.astype(BD)
    x_b = x.astype(BD)

    # rope tables, with 1/sqrt(sqrt(64)) on each side -> scores * 1/8
    inv_freq = 1.0 / (ROPE_BASE ** (np.arange(0, DH, 2, dtype=np.float32) / DH))
    tpos = np.arange(L, dtype=np.float32)
    freqs = np.einsum("l,f->lf", tpos, inv_freq)
    emb = np.concatenate([freqs, freqs], axis=-1)
    s8 = np.float32(8.0 ** -0.5)
    cos_t = (np.cos(emb) * s8).astype(np.float32)
    sin_t = (np.sin(emb) * s8).astype(np.float32)
    sinl = -sin_t[:, : DH // 2]
    sinh = sin_t[:, DH // 2 :]
    r1 = np.concatenate(
        [cos_t[:, : DH // 2] - sin_t[:, : DH // 2],
         cos_t[:, DH // 2 :] + sin_t[:, DH // 2 :]], axis=1
    )

    identf = np.eye(P, dtype=np.float32)
    identb = np.eye(P, dtype=BD)

    def wlay(a):   # [D, n] -> [P, KC, n] with d = c*P + p
        return np.ascontiguousarray(a.reshape(KC, P, -1).transpose(1, 0, 2))

    def tlay(a):   # [L, d] -> [P, TT, d] with l = n*P + p
        return np.ascontiguousarray(a.reshape(TT, P, -1).transpose(1, 0, 2))

    in_maps = []
    for c in range(8):
        b, g = c // HPC, c % HPC
        mine = np.arange(g * CD, (g + 1) * CD)

        sid = np.asarray(seq_id[b], np.int64)
        A = (sid[None, :] == np.arange(4)[:, None]).astype(np.float32)
        maskq = np.concatenate([MASK_A * A, MASK_A * np.ones((1, L), np.float32)])
        maskk = np.concatenate([MASK_A * A, -MASK_A * np.ones((1, L), np.float32)])

        m = {
            "x": np.ascontiguousarray(x_b[b]),
            "wq": wlay(wq_f[:, mine]),
            "wk": wlay(wk_f[:, mine]),
            "wv": wlay(wv_f[:, mine]),
            "wo": np.ascontiguousarray(
                wo_b[mine, :].reshape(CD // P, P, D).transpose(1, 0, 2)),
            "maskq": maskq.astype(BD),
            "maskk": maskk.astype(BD),
            "cos": tlay(cos_t.astype(BD)),
            "sinl": tlay(np.ascontiguousarray(sinl).astype(BD)),
            "sinh": tlay(np.ascontiguousarray(sinh).astype(BD)),
            "r1": tlay(np.ascontiguousarray(r1).astype(BD)),
            "identf": identf,
            "identb": identb,
        }
        if use_ln1b:
            m["lnb"] = ln1_b.reshape(1, D)
        if use_qlw:
            m["qlw"] = q_ln_w[mine].reshape(1, CD)
        if use_klw:
            m["klw"] = k_ln_w[mine].reshape(1, CD)
        in_maps.append(m)
    return in_maps, (use_ln1b, use_qlw, use_klw, qgr)


def run(inputs, trace=False):
    """Run on hardware; returns (output [B, L, D] fp32, BassKernelResults)."""
    in_maps, key = _host_prep(**inputs)
    nc = _get_nc(key)
    res = bass_utils.run_bass_kernel_spmd(
        nc, in_maps, core_ids=list(range(8)), trace=trace
    )
    out = np.zeros((B, L, D), np.float32)
    for c in range(8):
        out[c // HPC] += np.asarray(res.results[c]["out"], dtype=np.float32)
    return out, res


def kernel(**inputs) -> np.ndarray:
    out, _ = run(inputs)
    return out


# revision 38
# speedup vs baseline: 1.0904x; 1.0904x over previous
"""Trainium2 Bass kernel for MHA block (LN -> QKV -> qk-LN -> RoPE -> masked attn -> out-proj).

Self-contained: hardcodes shapes B=2, L=2048, D=1024, H=16, Dh=64; runs on 8 NeuronCores
via bass_utils.run_bass_kernel_spmd. Sharding: core c = (batch b = c//4, head-group
g = c%4 of 4 heads). Weight columns are sliced per core so "our" 4 heads are always
columns 0:256 -> the device program is identical on all cores (SPMD). The qk-LN
statistics (over the full 1024 dims) are formed from per-core partial sums with two
4-core-group AllReduces (split in halves to hide latency). RoPE is applied to the raw
q/k (it is linear) and the LN affine is folded in afterwards:
rot(LN(q)) = rstd*rot(q) - (rstd*mu)*rot(ones).

seq_id is sorted -> the attention mask is block diagonal. The host computes, per
256-query group, the key-tile range needed (union over the 2 batches so the SPMD
program stays shared) and the kernel only computes those (qgroup, key-tile) score/PV
units (~40% of dense). Attention is split into phase A (first-half queries x
first-half keys, runs while the 2nd stats AllReduce is in flight) and phase B (the
rest); partial context/denominator sums are additive, so phase A closes its partial
units into SBUF and phase B adds the remainder. Most operands are bf16 (matmul
accumulation stays fp32 in PSUM). Host sums the 4 partial out-projections per batch.
"""

import numpy as np
import ml_dtypes
from contextlib import ExitStack

import concourse.bass as bass
import concourse.tile as tile
from concourse import bacc, mybir
from concourse import bass_utils

F32 = mybir.dt.float32
BF16 = mybir.dt.bfloat16
AF = mybir.ActivationFunctionType
ALU = mybir.AluOpType

B, L, D = 2, 2048, 1024
H, DH = 16, 64
HPC = 4          # heads per core
CD = HPC * DH    # ctx dims per core = 256
P = 128
TT = L // P      # 16 token tiles
KC = D // P      # 8 contraction chunks
QG = 256         # query group width for block-sparse attention
NG = L // QG     # 8 query groups
EPS = 1e-5
ROPE_BASE = 10000.0
MASK_A = 8.0     # mask row scale; mask bias = -MASK_A^2 = -64 for masked pairs
KR = DH + 5      # contraction rows for scores (64 dims + 5 mask rows)
VB = DH + 1      # v block width (64 dims + ones col)
RG = [[0, 1, 2, 3], [4, 5, 6, 7]]


def _bcast_free(ap, n, axis):
    """Insert a step-0 free dim of size n at position `axis` (after partition dim)."""
    new = list(ap.ap)
    new.insert(axis, [0, n])
    return bass.AP(tensor=ap.tensor, offset=ap.offset, ap=new)


def _build_units(qgr):
    """Split (head, qgroup, key-tiles) into phase A (both halves finalized early)
    and phase B units. mode: 'copy' closes into craw, 'add' accumulates into it."""
    ua, ub = [], []
    for h in range(HPC):
        for g in range(NG):
            lo, hi = qgr[g]
            kts = list(range(lo, hi))
            if g < NG // 2:
                ka = [k for k in kts if k < TT // 2]
                kb = [k for k in kts if k >= TT // 2]
                if ka:
                    ua.append((h, g, ka, 'copy'))
                if kb:
                    ub.append((h, g, kb, 'add' if ka else 'copy'))
            else:
                ub.append((h, g, kts, 'copy'))
    return ua, ub


def build_bass(use_ln1b=False, use_qlw=False, use_klw=False,
               qgr=tuple((0, TT) for _ in range(NG))):
    nc = bacc.Bacc("TRN2", target_bir_lowering=False, debug=False, num_devices=8)
    use_lw = use_qlw or use_klw

    # ---- DRAM I/O ----
    x_d = nc.dram_tensor("x", [L, D], BF16, kind="ExternalInput").ap()
    wq_d = nc.dram_tensor("wq", [P, KC, CD], BF16, kind="ExternalInput").ap()
    wk_d = nc.dram_tensor("wk", [P, KC, CD], BF16, kind="ExternalInput").ap()
    wv_d = nc.dram_tensor("wv", [P, KC, CD], BF16, kind="ExternalInput").ap()
    wo_d = nc.dram_tensor("wo", [P, CD // P, D], BF16, kind="ExternalInput").ap()
    mq_d = nc.dram_tensor("maskq", [5, L], BF16, kind="ExternalInput").ap()
    mk_d = nc.dram_tensor("maskk", [5, L], BF16, kind="ExternalInput").ap()
    cos_d = nc.dram_tensor("cos", [P, TT, DH], BF16, kind="ExternalInput").ap()
    sinl_d = nc.dram_tensor("sinl", [P, TT, DH // 2], BF16, kind="ExternalInput").ap()
    sinh_d = nc.dram_tensor("sinh", [P, TT, DH // 2], BF16, kind="ExternalInput").ap()
    r1_d = nc.dram_tensor("r1", [P, TT, DH], BF16, kind="ExternalInput").ap()
    idf_d = nc.dram_tensor("identf", [P, P], F32, kind="ExternalInput").ap()
    idb_d = nc.dram_tensor("identb", [P, P], BF16, kind="ExternalInput").ap()
    if use_ln1b:
        lnb_d = nc.dram_tensor("lnb", [1, D], F32, kind="ExternalInput").ap()
    if use_qlw:
        qlw_d = nc.dram_tensor("qlw", [1, CD], F32, kind="ExternalInput").ap()
    if use_klw:
        klw_d = nc.dram_tensor("klw", [1, CD], F32, kind="ExternalInput").ap()
    out_d = nc.dram_tensor("out", [L, D], BF16, kind="ExternalOutput").ap()

    x_t_d = x_d.rearrange("(n p) d -> n p d", p=P)
    out_t_d = out_d.rearrange("(n p) d -> n p d", p=P)

    units_a, units_b = _build_units(qgr)

    with tile.TileContext(nc) as tc, ExitStack() as ctx:
        cpool = ctx.enter_context(tc.tile_pool(name="cpool", bufs=1))
        small = ctx.enter_context(tc.tile_pool(name="small", bufs=4))

        identb = cpool.tile([P, P], BF16)
        nc.sync.dma_start(identb, idb_d)
        identf = cpool.tile([P, P], F32)
        eps_ap = cpool.tile([P, 1], F32)
        nc.vector.memset(eps_ap, EPS)

        # v augmented: flat [128, TT*HPC*65 + 63]; per (kt,h) block of 65 cols
        # (64 v dims + ones col). PV reads 128 cols per block: the 63 cols past a
        # block belong to the next block -> garbage rows 65:128 in ctx psum, unread.
        pB = ctx.enter_context(tc.tile_pool(name="pB", bufs=1))
        v_sb = pB.tile([P, TT * HPC * VB + (P - VB)], BF16)
        v_blocks = v_sb[:, : TT * HPC * VB].rearrange("p (t h d) -> p t h d", t=TT, h=HPC)
        nc.gpsimd.memset(v_blocks[:, :, :, DH : DH + 1], 1.0)

        # qT/kT augmented per head: rows 0:64 = head dims (transposed), 64:69 = mask
        # rows -> scores+mask in ONE matmul over 69 contraction rows.
        qT = pB.tile([P, HPC, L], BF16)
        kT = pB.tile([P, HPC, L], BF16)
        # rope'd+LN-finalized q/k in token-major bf16, awaiting transpose
        rotb_q = pB.tile([P, TT, HPC, DH], BF16)
        rotb_k = pB.tile([P, TT, HPC, DH], BF16)
        craw_all = pB.tile([DH + 1, HPC, L], BF16)
        ctxT = pB.tile([P, CD // P, L], BF16)

        def rotb_at(j, t):
            return (rotb_q if j == 0 else rotb_k)[:, t, :, :]

        # ============ Phase 1: LN1 + QKV (our 768 cols) + partial stats + raw rope
        with ExitStack() as phA:
            pA = phA.enter_context(tc.tile_pool(name="pA", bufs=1))
            stats_pack = pA.tile([P, TT, 2, 2], F32)
            allred = pA.tile([P, TT, 2, 2], F32)
            rot_q = pA.tile([P, TT, HPC, DH], F32)
            rot_k = pA.tile([P, TT, HPC, DH], F32)

            def rot_at(j, t):
                return (rot_q if j == 0 else rot_k)[:, t, :, :]

            wq_sb = pA.tile([P, KC, CD], BF16)
            wk_sb = pA.tile([P, KC, CD], BF16)
            wv_sb = pA.tile([P, KC, CD], BF16)
            r1_sb = pA.tile([P, TT, DH], BF16)
            if use_lw:
                q4_all = pA.tile([P, TT, CD], F32)
                k4_all = pA.tile([P, TT, CD], F32)
                cos2_sb = pA.tile([P, TT, DH], BF16)
                nc.sync.dma_start(cos2_sb, cos_d)
                sinl2_sb = pA.tile([P, TT, DH // 2], BF16)
                nc.sync.dma_start(sinl2_sb, sinl_d)
                sinh2_sb = pA.tile([P, TT, DH // 2], BF16)
                nc.sync.dma_start(sinh2_sb, sinh_d)
                if use_qlw:
                    qlw_sb = pA.tile([P, CD], F32)
                    nc.sync.dma_start(qlw_sb, qlw_d.partition_broadcast(P)[:, 0, :])
                if use_klw:
                    klw_sb = pA.tile([P, CD], F32)
                    nc.sync.dma_start(klw_sb, klw_d.partition_broadcast(P)[:, 0, :])
            dramp = phA.enter_context(tc.tile_pool(name="dramp", bufs=1, space="DRAM"))
            ib1 = dramp.tile([P, TT * 2], F32)
            ob1 = dramp.tile([P, TT * 2], F32)
            ib2 = dramp.tile([P, TT * 2], F32)
            ob2 = dramp.tile([P, TT * 2], F32)

            def finalize_apply(lo, hi, after_group=None):
                """qk-LN: mu/rstd from all-reduced sums, fold into raw-rope'd q/k.
                rotb = rstd * (rot + (-mu) * r1): one DVE STT + one ACT scale per
                (tile, q/k). after_group(g4) fires once each 4-tile group is done."""
                n = hi - lo
                mu = small.tile([P, n, 2], F32, tag="fmu")
                nc.vector.tensor_scalar(mu, allred[:, lo:hi, :, 0], 1.0 / D, None, ALU.mult)
                m2 = small.tile([P, n, 2], F32, tag="fm2")
                nc.vector.tensor_mul(m2, mu, mu)
                rstd = small.tile([P, n, 2], F32, tag="frstd")
                nc.vector.scalar_tensor_tensor(
                    rstd, allred[:, lo:hi, :, 1], 1.0 / D, m2,
                    op0=ALU.mult, op1=ALU.subtract,
                )
                nc.scalar.activation(rstd, rstd, AF.Sqrt, bias=eps_ap)
                nc.vector.reciprocal(rstd, rstd)
                negmu = small.tile([P, n, 2], F32, tag="fnegmu")
                nc.vector.tensor_scalar(negmu, mu, -1.0, None, ALU.mult)
                nm = small.tile([P, n, 2], F32, tag="fnm")
                if use_lw:
                    nc.vector.scalar_tensor_tensor(nm, mu, -1.0, rstd, op0=ALU.mult, op1=ALU.mult)
                for t in range(lo, hi):
                    for j in range(2):
                        lw_flag = use_qlw if j == 0 else use_klw
                        if lw_flag:
                            src4 = q4_all if j == 0 else k4_all
                            lw_sb = qlw_sb if j == 0 else klw_sb
                            xn = small.tile([P, HPC, DH], F32, tag="xnf")
                            nc.scalar.activation(
                                xn.rearrange("p h d -> p (h d)"), src4[:, t, :],
                                AF.Identity, bias=nm[:, t - lo, j : j + 1],
                                scale=rstd[:, t - lo, j : j + 1],
                            )
                            nc.vector.tensor_mul(
                                xn, xn, lw_sb.rearrange("p (h d) -> p h d", h=HPC)
                            )
                            qa2 = small.tile([P, HPC, DH], F32, tag="qa2")
                            nc.vector.tensor_mul(
                                qa2, xn, _bcast_free(cos2_sb[:, t, :], HPC, 1)
                            )
                            qb2 = small.tile([P, HPC, DH], F32, tag="qb2")
                            nc.vector.tensor_mul(
                                qb2[:, :, 0 : DH // 2], xn[:, :, DH // 2 : DH],
                                _bcast_free(sinl2_sb[:, t, :], HPC, 1),
                            )
                            nc.vector.tensor_mul(
                                qb2[:, :, DH // 2 : DH], xn[:, :, 0 : DH // 2],
                                _bcast_free(sinh2_sb[:, t, :], HPC, 1),
                            )
                            nc.vector.tensor_add(rotb_at(j, t), qa2, qb2)
                            continue
                        tmp = small.tile([P, HPC, DH], F32, tag=f"tmp{j}", bufs=2)
                        nc.vector.scalar_tensor_tensor(
                            tmp, _bcast_free(r1_sb[:, t, :], HPC, 1),
                            negmu[:, t - lo, j : j + 1], rot_at(j, t),
                            op0=ALU.mult, op1=ALU.add,
                        )
                        nc.scalar.activation(
                            rotb_at(j, t).rearrange("p h d -> p (h d)"),
                            tmp.rearrange("p h d -> p (h d)"),
                            AF.Identity, scale=rstd[:, t - lo, j : j + 1],
                        )
                    if after_group is not None and t % 4 == 3:
                        after_group(t // 4)

            with ExitStack() as pctx:
                pp_qkv = pctx.enter_context(tc.tile_pool(name="pp_qkv", bufs=6, space="PSUM"))
                pp_ht = pctx.enter_context(tc.tile_pool(name="pp_ht", bufs=2, space="PSUM"))
                p1 = pctx.enter_context(tc.tile_pool(name="p1", bufs=2))

                x_pre = []
                for t in range(2):
                    x_t = p1.tile([P, D], BF16, tag="x_t", bufs=16, name=f"x_pre{t}")
                    nc.sync.dma_start(x_t, x_t_d[t])
                    x_pre.append(x_t)
                nc.sync.dma_start(wq_sb, wq_d)
                for t in range(2, 4):
                    x_t = p1.tile([P, D], BF16, tag="x_t", bufs=16, name=f"x_pre{t}")
                    nc.sync.dma_start(x_t, x_t_d[t])
                    x_pre.append(x_t)
                nc.sync.dma_start(wk_sb, wk_d)
                cos_sb = p1.tile([P, TT, DH], BF16, bufs=1)
                nc.sync.dma_start(cos_sb, cos_d)
                sinl_sb = p1.tile([P, TT, DH // 2], BF16, bufs=1)
                nc.sync.dma_start(sinl_sb, sinl_d)
                sinh_sb = p1.tile([P, TT, DH // 2], BF16, bufs=1)
                nc.sync.dma_start(sinh_sb, sinh_d)
                nc.sync.dma_start(wv_sb, wv_d)
                for t in range(4, 8):
                    x_t = p1.tile([P, D], BF16, tag="x_t", bufs=16, name=f"x_pre{t}")
                    nc.sync.dma_start(x_t, x_t_d[t])
                    x_pre.append(x_t)
                nc.sync.dma_start(r1_sb, r1_d)
                for t in range(8, TT):
                    x_t = p1.tile([P, D], BF16, tag="x_t", bufs=16, name=f"x_pre{t}")
                    nc.sync.dma_start(x_t, x_t_d[t])
                    x_pre.append(x_t)
                if use_ln1b:
                    lnb_sb = p1.tile([P, D], F32, bufs=1)
                    nc.sync.dma_start(lnb_sb, lnb_d.partition_broadcast(P)[:, 0, :])

                def stage1(t):
                    """LN1 + hT (DMA transpose) + QKV matmuls."""
                    x_t = x_pre[t]

                    xstats = small.tile([P, 2, 6], F32, tag="xstats")
                    for s in range(2):
                        nc.vector.bn_stats(
                            xstats[:, s, :],
                            x_t[:, s * 512 : (s + 1) * 512].rearrange(
                                "p (s d) -> p s d", s=1
                            ),
                        )
                    xmv = small.tile([P, 2], F32, tag="xmv")
                    nc.vector.bn_aggr(xmv, xstats)
                    xrstd = small.tile([P, 1], F32, tag="xrstd")
                    nc.scalar.activation(xrstd, xmv[:, 1:2], AF.Sqrt, bias=eps_ap)
                    nc.vector.reciprocal(xrstd, xrstd)
                    xnm = small.tile([P, 1], F32, tag="xnm")
                    nc.vector.tensor_scalar(xnm, xmv[:, 0:1], xrstd, -1.0, ALU.mult, ALU.mult)
                    h_t = p1.tile([P, D], BF16, tag="h_t", bufs=3)
                    nc.scalar.activation(h_t, x_t, AF.Identity, bias=xnm, scale=xrstd)
                    if use_ln1b:
                        nc.vector.tensor_add(h_t, h_t, lnb_sb)

                    ht_ps = pp_ht.tile([P, KC, P], BF16, tag="ht")
                    for c in range(KC):
                        nc.tensor.transpose(ht_ps[:, c, :], h_t[:, c * P : (c + 1) * P], identb)
                    hT_t = p1.tile([P, KC, P], BF16, tag="hT_t", bufs=2)
                    nc.scalar.copy(hT_t, ht_ps)

                    q_ps = pp_qkv.tile([P, CD], F32, tag="qkv", name="q_ps")
                    k_ps = pp_qkv.tile([P, CD], F32, tag="qkv", name="k_ps")
                    v_ps = pp_qkv.tile([P, CD], F32, tag="qkv", name="v_ps")
                    for c in range(KC):
                        nc.tensor.matmul(q_ps, hT_t[:, c, :], wq_sb[:, c, :],
                                         start=(c == 0), stop=(c == KC - 1))
                    for c in range(KC):
                        nc.tensor.matmul(k_ps, hT_t[:, c, :], wk_sb[:, c, :],
                                         start=(c == 0), stop=(c == KC - 1))
                    for c in range(KC):
                        nc.tensor.matmul(v_ps, hT_t[:, c, :], wv_sb[:, c, :],
                                         start=(c == 0), stop=(c == KC - 1))
                    return t, q_ps, k_ps, v_ps

                def stage2(st):
                    """Drain q/k/v psums, stats accums, raw rope."""
                    t, q_ps, k_ps, v_ps = st
                    if use_lw:
                        q4_t = q4_all[:, t, :]
                        k4_t = k4_all[:, t, :]
                    else:
                        q4_t = p1.tile([P, CD], F32, tag="q4t", bufs=3)
                        k4_t = p1.tile([P, CD], F32, tag="k4t", bufs=3)
                    nc.scalar.activation(
                        q4_t, q_ps, AF.Copy, accum_out=stats_pack[:, t, 0, 0:1]
                    )
                    nc.scalar.activation(
                        k4_t, k_ps, AF.Copy, accum_out=stats_pack[:, t, 1, 0:1]
                    )
                    nc.vector.tensor_copy(
                        v_blocks[:, t, :, 0:DH],
                        v_ps.rearrange("p (h d) -> p h d", h=HPC),
                    )
                    # s2 = rowsum(q4^2) on DVE, from SBUF
                    sq = p1.tile([P, CD], F32, tag="sq", bufs=2)
                    nc.vector.scalar_tensor_tensor(
                        sq, q4_t, 1.0, q4_t, op0=ALU.mult, op1=ALU.mult,
                        accum_out=stats_pack[:, t, 0, 1:2],
                    )
                    nc.vector.scalar_tensor_tensor(
                        sq, k4_t, 1.0, k4_t, op0=ALU.mult, op1=ALU.mult,
                        accum_out=stats_pack[:, t, 1, 1:2],
                    )

                    # raw rope (linear; LN affine folded in afterwards), from SBUF
                    for j in range(2):
                        src4 = q4_t if j == 0 else k4_t
                        xn4 = src4.rearrange("p (h d) -> p h d", h=HPC)
                        qa = p1.tile([P, HPC, DH], F32, tag="qa", bufs=2)
                        nc.vector.tensor_mul(qa, xn4, _bcast_free(cos_sb[:, t, :], HPC, 1))
                        qb = p1.tile([P, HPC, DH], F32, tag="qb", bufs=2)
                        nc.gpsimd.tensor_mul(
                            qb[:, :, 0 : DH // 2],
                            xn4[:, :, DH // 2 : DH],
                            _bcast_free(sinl_sb[:, t, :], HPC, 1),
                        )
                        nc.gpsimd.tensor_mul(
                            qb[:, :, DH // 2 : DH],
                            xn4[:, :, 0 : DH // 2],
                            _bcast_free(sinh_sb[:, t, :], HPC, 1),
                        )
                        nc.vector.tensor_add(rot_at(j, t), qa, qb)

                # two-stage software pipeline: stage1(t) runs one tile ahead of
                # stage2(t-1) so PE never waits on the psum-drain/rope tail.
                st_prev = None
                for t in range(TT):
                    st = stage1(t)
                    if st_prev is not None:
                        stage2(st_prev)
                        if t == 8:
                            # first-half AllReduce; hides under tiles 8-15
                            nc.gpsimd.dma_start(
                                ib1[:],
                                stats_pack[:, 0:8, :, :].rearrange("p t j s -> p (t j s)"),
                            )
                            nc.gpsimd.collective_compute(
                                "AllReduce", ALU.add, replica_groups=RG,
                                ins=[ib1.opt()], outs=[ob1.opt()],
                            )
                            nc.sync.dma_start(
                                allred[:, 0:8, :, :].rearrange("p t j s -> p (t j s)"),
                                ob1[:],
                            )
                    st_prev = st
                stage2(st_prev)

            # late constants: mask rows + fp32 identity (not needed until here)
            for hh in range(HPC):
                nc.sync.dma_start(qT[DH : DH + 5, hh, :], mq_d)
                nc.sync.dma_start(kT[DH : DH + 5, hh, :], mk_d)
            nc.sync.dma_start(identf, idf_d)

            # second-half AllReduce launched immediately; its latency is hidden
            # under finalize(0:8) + first-half transposes + phase-A attention.
            nc.gpsimd.dma_start(
                ib2[:],
                stats_pack[:, 8:16, :, :].rearrange("p t j s -> p (t j s)"),
            )
            nc.gpsimd.collective_compute(
                "AllReduce", ALU.add, replica_groups=RG,
                ins=[ib2.opt()], outs=[ob2.opt()],
            )
            nc.sync.dma_start(
                allred[:, 8:16, :, :].rearrange("p t j s -> p (t j s)"),
                ob2[:],
            )

            # ============ Phase 2: attention (A while AR2 in flight, then B)
            with ExitStack() as actx:
                pa_sc = actx.enter_context(tc.tile_pool(name="pa_sc", bufs=3, space="PSUM"))
                pa_ctx = actx.enter_context(tc.tile_pool(name="pa_ctx", bufs=3, space="PSUM"))
                pp_tr = actx.enter_context(tc.tile_pool(name="pp_tr", bufs=2, space="PSUM"))
                p2 = actx.enter_context(tc.tile_pool(name="p2", bufs=4))

                def emit_transposes_group(g4):
                    for j in range(2):
                        dst = qT if j == 0 else kT
                        for hh in range(HPC):
                            tp = pp_tr.tile([DH, 4, P], BF16, tag="tp",
                                            name=f"tp{j}{hh}{g4}")
                            for i in range(4):
                                nc.tensor.transpose(
                                    tp[:, i, :], rotb_at(j, g4 * 4 + i)[:, hh, :],
                                    identb,
                                )
                            if hh % 2 == 0:
                                nc.scalar.copy(
                                    dst[0:DH, hh, g4 * 512 : (g4 + 1) * 512],
                                    tp.rearrange("p g t -> p (g t)"),
                                )
                            else:
                                nc.vector.tensor_copy(
                                    dst[0:DH, hh, g4 * 512 : (g4 + 1) * 512],
                                    tp.rearrange("p g t -> p (g t)"),
                                )

                def emit_attn(units):
                    # flatten to chunks of <=2 key tiles; 2-chunk software pipeline
                    chunks = []
                    for ui, (h, g, kts, mode) in enumerate(units):
                        for ci in range(0, len(kts), 2):
                            sub = kts[ci : ci + 2]
                            chunks.append({
                                "h": h, "g": g, "kts": sub, "mode": mode, "ui": ui,
                                "first": ci == 0, "last": ci + 2 >= len(kts),
                            })
                    cur_ctx = [None]

                    def emit_pv(c):
                        nk = len(c["kts"])
                        for i, kt in enumerate(c["kts"]):
                            voff = (kt * HPC + c["h"]) * VB
                            nc.tensor.matmul(
                                c["ctx"], v_sb[:, voff : voff + P],
                                c["eT"][:, i * QG : (i + 1) * QG],
                                start=(c["first"] and i == 0),
                                stop=(c["last"] and i == nk - 1),
                            )
                        if c["last"]:
                            dst = craw_all[:, c["h"], c["g"] * QG : (c["g"] + 1) * QG]
                            if c["mode"] == "add":
                                nc.vector.tensor_add(dst, dst, c["ctx"][0 : DH + 1, :])
                            else:
                                nc.vector.tensor_copy(dst, c["ctx"][0 : DH + 1, :])

                    pend = []
                    for c in chunks:
                        nk = len(c["kts"])
                        s_ps = pa_sc.tile([P, 2 * QG], F32, tag="sc", name="s_ps")
                        for i, kt in enumerate(c["kts"]):
                            nc.tensor.matmul(
                                s_ps[:, i * QG : (i + 1) * QG],
                                kT[0:KR, c["h"], kt * P : (kt + 1) * P],
                                qT[0:KR, c["h"], c["g"] * QG : (c["g"] + 1) * QG],
                                start=True, stop=True,
                            )
                        eT = p2.tile([P, 2 * QG], BF16, tag="eT", name="eT")
                        nc.scalar.activation(
                            eT[:, : nk * QG], s_ps[:, : nk * QG], AF.Exp
                        )
                        c["eT"] = eT
                        if c["first"]:
                            cur_ctx[0] = pa_ctx.tile([P, QG], F32, tag="ctx",
                                                     name="ctx_ps", bufs=3)
                        c["ctx"] = cur_ctx[0]
                        pend.append(c)
                        if len(pend) > 2:
                            emit_pv(pend.pop(0))
                    for c in pend:
                        emit_pv(c)

                wo_sb = p2.tile([P, CD // P, D], BF16, tag="wo", bufs=1)
                nc.sync.dma_start(wo_sb, wo_d)

                def emit_outproj_grp(grp):
                    """Normalize craw for qgroups 2*grp..2*grp+1 and project out."""
                    for h in range(HPC):
                        pr, sub = h // 2, h % 2
                        rows = slice(sub * DH, (sub + 1) * DH)
                        fwd = pp_tr.tile([P, 4, DH + 2], BF16, tag="tp",
                                         name=f"fwd{grp}{h}")
                        for i in range(4):
                            tix = grp * 4 + i
                            nc.tensor.transpose(
                                fwd[:, i, 0 : DH + 1],
                                craw_all[:, h, tix * P : (tix + 1) * P],
                                identb[: DH + 1, : DH + 1],
                            )
                        rz = small.tile([P, 4], F32, tag="rz")
                        nc.vector.reciprocal(rz, fwd[:, :, DH])
                        cn = p2.tile([P, 4, DH], BF16, tag="cn", bufs=2,
                                     name=f"cn{grp}{h}")
                        nc.vector.tensor_mul(cn, fwd[:, :, 0:DH], _bcast_free(rz, DH, 2))
                        back = pp_tr.tile([DH, 4, P], BF16, tag="tp",
                                          name=f"back{grp}{h}")
                        for i in range(4):
                            nc.tensor.transpose(back[:, i, :], cn[:, i, :], identb)
                        nc.vector.tensor_copy(
                            ctxT[rows, pr, grp * 512 : (grp + 1) * 512],
                            back.rearrange("p g t -> p (g t)"),
                        )
                    for i in range(4):
                        t = grp * 4 + i
                        o_ps = [pa_ctx.tile([P, 512], F32, tag="ctx", bufs=3,
                                            name=f"o_ps{t}{s}")
                                for s in range(2)]
                        for s in range(2):
                            for c in range(CD // P):
                                nc.tensor.matmul(
                                    o_ps[s], ctxT[:, c, t * P : (t + 1) * P],
                                    wo_sb[:, c, s * 512 : (s + 1) * 512],
                                    start=(c == 0), stop=(c == CD // P - 1),
                                )
                        o_t = p2.tile([P, D], BF16, tag="o_t", bufs=2, name=f"o_t{t}")
                        nc.scalar.copy(o_t[:, 0:512], o_ps[0])
                        nc.vector.tensor_copy(o_t[:, 512:1024], o_ps[1])
                        nc.sync.dma_start(out_t_d[t], o_t)

                finalize_apply(0, 8, after_group=emit_transposes_group)
                emit_attn(units_a)
                finalize_apply(8, 16, after_group=emit_transposes_group)
                # phase B grouped by qgroup pairs so each outproj group starts
                # as soon as its craw columns close
                ub = sorted(units_b, key=lambda u: (u[1], u[0]))
                for grp in range(4):
                    gs = (2 * grp, 2 * grp + 1)
                    emit_attn([u for u in ub if u[1] in gs])
                    emit_outproj_grp(grp)

    nc.compile()
    return nc


_CACHE = {}


def _get_nc(key):
    if key not in _CACHE:
        _CACHE[key] = build_bass(*key)
    return _CACHE[key]


def _qg_ranges(seq_id):
    """Per 256-query group: key-tile range (lo, hi) needed, unioned over batches.
    Falls back to dense if any row is unsorted."""
    for b in range(seq_id.shape[0]):
        if np.any(np.diff(seq_id[b].astype(np.int64)) < 0):
            return tuple((0, TT) for _ in range(NG))
    rs = []
    for g in range(NG):
        lo, hi = L, 0
        for b in range(seq_id.shape[0]):
            s = seq_id[b]
            l = int(np.searchsorted(s, s[g * QG], 'left'))
            h_ = int(np.searchsorted(s, s[(g + 1) * QG - 1], 'right'))
            lo = min(lo, l)
            hi = max(hi, h_)
        rs.append((lo // P, -(-hi // P)))
    return tuple(rs)


def _host_prep(x, seq_id, ln1_w, ln1_b, w_qkv, q_ln_w, k_ln_w, w_out):
    """Build the 8 per-core input maps."""
    x = np.asarray(x, np.float32)
    seq_id = np.asarray(seq_id)
    ln1_w = np.asarray(ln1_w, np.float32)
    ln1_b = np.asarray(ln1_b, np.float32)
    w_qkv = np.asarray(w_qkv, np.float32)
    q_ln_w = np.asarray(q_ln_w, np.float32)
    k_ln_w = np.asarray(k_ln_w, np.float32)
    w_out = np.asarray(w_out, np.float32)

    use_ln1b = bool(np.any(ln1_b != 0.0))
    use_qlw = not np.allclose(q_ln_w, 1.0)
    use_klw = not np.allclose(k_ln_w, 1.0)
    qgr = _qg_ranges(seq_id)

    BD = ml_dtypes.bfloat16
    wq_f = (w_qkv[:, 0:D] * ln1_w[:, None]).astype(BD)
    wk_f = (w_qkv[:, D : 2 * D] * ln1_w[:, None]).astype(BD)
    wv_f = (w_qkv[:, 2 * D : 3 * D] * ln1_w[:, None]).astype(BD)
    wo_b = w_out.astype(BD)
    x_b = x.astype(BD)

    # rope tables, with 1/sqrt(sqrt(64)) on each side -> scores * 1/8
    inv_freq = 1.0 / (ROPE_BASE ** (np.arange(0, DH, 2, dtype=np.float32) / DH))
    tpos = np.arange(L, dtype=np.float32)
    freqs = np.einsum("l,f->lf", tpos, inv_freq)
    emb = np.concatenate([freqs, freqs], axis=-1)
    s8 = np.float32(8.0 ** -0.5)
    cos_t = (np.cos(emb) * s8).astype(np.float32)
    sin_t = (np.sin(emb) * s8).astype(np.float32)
    sinl = -sin_t[:, : DH // 2]
    sinh = sin_t[:, DH // 2 :]
    r1 = np.concatenate(
        [cos_t[:, : DH // 2] - sin_t[:, : DH // 2],
         cos_t[:, DH // 2 :] + sin_t[:, DH // 2 :]], axis=1
    )

    identf = np.eye(P, dtype=np.float32)
    identb = np.eye(P, dtype=BD)

    def wlay(a):   # [D, n] -> [P, KC, n] with d = c*P + p
        return np.ascontiguousarray(a.reshape(KC, P, -1).transpose(1, 0, 2))

    def tlay(a):   # [L, d] -> [P, TT, d] with l = n*P + p
        return np.ascontiguousarray(a.reshape(TT, P, -1).transpose(1, 0, 2))

    in_maps = []
    for c in range(8):
        b, g = c // HPC, c % HPC
        mine = np.arange(g * CD, (g + 1) * CD)

        sid = np.asarray(seq_id[b], np.int64)
        A = (sid[None, :] == np.arange(4)[:, None]).astype(np.float32)
        maskq = np.concatenate([MASK_A * A, MASK_A * np.ones((1, L), np.float32)])
        maskk = np.concatenate([MASK_A * A, -MASK_A * np.ones((1, L), np.float32)])

        m = {
            "x": np.ascontiguousarray(x_b[b]),
            "wq": wlay(wq_f[:, mine]),
            "wk": wlay(wk_f[:, mine]),
            "wv": wlay(wv_f[:, mine]),
            "wo": np.ascontiguousarray(
                wo_b[mine, :].reshape(CD // P, P, D).transpose(1, 0, 2)),
            "maskq": maskq.astype(BD),
            "maskk": maskk.astype(BD),
            "cos": tlay(cos_t.astype(BD)),
            "sinl": tlay(np.ascontiguousarray(sinl).astype(BD)),
            "sinh": tlay(np.ascontiguousarray(sinh).astype(BD)),
            "r1": tlay(np.ascontiguousarray(r1).astype(BD)),
            "identf": identf,
            "identb": identb,
        }
        if use_ln1b:
            m["lnb"] = ln1_b.reshape(1, D)
        if use_qlw:
            m["qlw"] = q_ln_w[mine].reshape(1, CD)
        if use_klw:
            m["klw"] = k_ln_w[mine].reshape(1, CD)
        in_maps.append(m)
    return in_maps, (use_ln1b, use_qlw, use_klw, qgr)


def run(inputs, trace=False):
    """Run on hardware; returns (output [B, L, D] fp32, BassKernelResults)."""
    in_maps, key = _host_prep(**inputs)
    nc = _get_nc(key)
    res = bass_utils.run_bass_kernel_spmd(
        nc, in_maps, core_ids=list(range(8)), trace=trace
    )
    out = np.zeros((B, L, D), np.float32)
    for c in range(8):
        out[c // HPC] += np.asarray(res.results[c]["out"], dtype=np.float32)
    return out, res


def kernel(**inputs) -> np.ndarray:
    out, _ = run(inputs)
    return out


# revision 39
# speedup vs baseline: 1.1070x; 1.0152x over previous
"""Trainium2 Bass kernel for MHA block (LN -> QKV -> qk-LN -> RoPE -> masked attn -> out-proj).

Self-contained: hardcodes shapes B=2, L=2048, D=1024, H=16, Dh=64; runs on 8 NeuronCores
via bass_utils.run_bass_kernel_spmd. Sharding: core c = (batch b = c//4, head-group
g = c%4 of 4 heads). Weight columns are sliced per core so "our" 4 heads are always
columns 0:256 -> the device program is identical on all cores (SPMD). The qk-LN
statistics (over the full 1024 dims) are formed from per-core partial sums with two
4-core-group AllReduces (split in halves to hide latency). RoPE is applied to the raw
q/k (it is linear) and the LN affine is folded in afterwards:
rot(LN(q)) = rstd*rot(q) - (rstd*mu)*rot(ones).

seq_id is sorted -> the attention mask is block diagonal. The host computes, per
256-query group, the key-tile range needed (union over the 2 batches so the SPMD
program stays shared) and the kernel only computes those (qgroup, key-tile) score/PV
units (~40% of dense). Attention is split into phase A (first-half queries x
first-half keys, runs while the 2nd stats AllReduce is in flight) and phase B (the
rest); partial context/denominator sums are additive, so phase A closes its partial
units into SBUF and phase B adds the remainder. Most operands are bf16 (matmul
accumulation stays fp32 in PSUM). Host sums the 4 partial out-projections per batch.
"""

import numpy as np
import ml_dtypes
from contextlib import ExitStack

import concourse.bass as bass
import concourse.tile as tile
from concourse import bacc, mybir
from concourse import bass_utils

F32 = mybir.dt.float32
BF16 = mybir.dt.bfloat16
AF = mybir.ActivationFunctionType
ALU = mybir.AluOpType

B, L, D = 2, 2048, 1024
H, DH = 16, 64
HPC = 4          # heads per core
CD = HPC * DH    # ctx dims per core = 256
P = 128
TT = L // P      # 16 token tiles
KC = D // P      # 8 contraction chunks
QG = 256         # query group width for block-sparse attention
NG = L // QG     # 8 query groups
EPS = 1e-5
ROPE_BASE = 10000.0
MASK_A = 8.0     # mask row scale; mask bias = -MASK_A^2 = -64 for masked pairs
KR = DH + 5      # contraction rows for scores (64 dims + 5 mask rows)
VB = DH + 1      # v block width (64 dims + ones col)
RG = [[0, 1, 2, 3], [4, 5, 6, 7]]


def _bcast_free(ap, n, axis):
    """Insert a step-0 free dim of size n at position `axis` (after partition dim)."""
    new = list(ap.ap)
    new.insert(axis, [0, n])
    return bass.AP(tensor=ap.tensor, offset=ap.offset, ap=new)


def _build_units(qgr):
    """Split (head, qgroup, key-tiles) into phase A (both halves finalized early)
    and phase B units. mode: 'copy' closes into craw, 'add' accumulates into it."""
    ua, ub = [], []
    for h in range(HPC):
        for g in range(NG):
            lo, hi = qgr[g]
            kts = list(range(lo, hi))
            if g < NG // 2:
                ka = [k for k in kts if k < TT // 2]
                kb = [k for k in kts if k >= TT // 2]
                if ka:
                    ua.append((h, g, ka, 'copy'))
                if kb:
                    ub.append((h, g, kb, 'add' if ka else 'copy'))
            else:
                ub.append((h, g, kts, 'copy'))
    return ua, ub


def build_bass(use_ln1b=False, use_qlw=False, use_klw=False,
               qgr=tuple((0, TT) for _ in range(NG))):
    nc = bacc.Bacc("TRN2", target_bir_lowering=False, debug=False, num_devices=8)
    use_lw = use_qlw or use_klw

    # ---- DRAM I/O ----
    x_d = nc.dram_tensor("x", [L, D], BF16, kind="ExternalInput").ap()
    wq_d = nc.dram_tensor("wq", [P, KC, CD], BF16, kind="ExternalInput").ap()
    wk_d = nc.dram_tensor("wk", [P, KC, CD], BF16, kind="ExternalInput").ap()
    wv_d = nc.dram_tensor("wv", [P, KC, CD], BF16, kind="ExternalInput").ap()
    wo_d = nc.dram_tensor("wo", [P, CD // P, D], BF16, kind="ExternalInput").ap()
    mq_d = nc.dram_tensor("maskq", [5, L], BF16, kind="ExternalInput").ap()
    mk_d = nc.dram_tensor("maskk", [5, L], BF16, kind="ExternalInput").ap()
    cos_d = nc.dram_tensor("cos", [P, TT, DH], BF16, kind="ExternalInput").ap()
    sinl_d = nc.dram_tensor("sinl", [P, TT, DH // 2], BF16, kind="ExternalInput").ap()
    sinh_d = nc.dram_tensor("sinh", [P, TT, DH // 2], BF16, kind="ExternalInput").ap()
    r1_d = nc.dram_tensor("r1", [P, TT, DH], BF16, kind="ExternalInput").ap()
    idb_d = nc.dram_tensor("identb", [P, P], BF16, kind="ExternalInput").ap()
    if use_ln1b:
        lnb_d = nc.dram_tensor("lnb", [1, D], F32, kind="ExternalInput").ap()
    if use_qlw:
        qlw_d = nc.dram_tensor("qlw", [1, CD], F32, kind="ExternalInput").ap()
    if use_klw:
        klw_d = nc.dram_tensor("klw", [1, CD], F32, kind="ExternalInput").ap()
    out_d = nc.dram_tensor("out", [L, D], BF16, kind="ExternalOutput").ap()

    x_t_d = x_d.rearrange("(n p) d -> n p d", p=P)
    out_t_d = out_d.rearrange("(n p) d -> n p d", p=P)

    units_a, units_b = _build_units(qgr)

    with tile.TileContext(nc) as tc, ExitStack() as ctx:
        cpool = ctx.enter_context(tc.tile_pool(name="cpool", bufs=1))
        small = ctx.enter_context(tc.tile_pool(name="small", bufs=4))

        identb = cpool.tile([P, P], BF16)
        nc.sync.dma_start(identb, idb_d)
        eps_ap = cpool.tile([P, 1], F32)
        nc.vector.memset(eps_ap, EPS)

        # v augmented: flat [128, TT*HPC*65 + 63]; per (kt,h) block of 65 cols
        # (64 v dims + ones col). PV reads 128 cols per block: the 63 cols past a
        # block belong to the next block -> garbage rows 65:128 in ctx psum, unread.
        pB = ctx.enter_context(tc.tile_pool(name="pB", bufs=1))
        v_sb = pB.tile([P, TT * HPC * VB + (P - VB)], BF16)
        v_blocks = v_sb[:, : TT * HPC * VB].rearrange("p (t h d) -> p t h d", t=TT, h=HPC)
        nc.gpsimd.memset(v_blocks[:, :, :, DH : DH + 1], 1.0)

        # qT/kT augmented per head: rows 0:64 = head dims (transposed), 64:69 = mask
        # rows -> scores+mask in ONE matmul over 69 contraction rows.
        qT = pB.tile([P, HPC, L], BF16)
        kT = pB.tile([P, HPC, L], BF16)
        # rope'd+LN-finalized q/k in token-major bf16, awaiting transpose
        rotb_q = pB.tile([P, TT, HPC, DH], BF16)
        rotb_k = pB.tile([P, TT, HPC, DH], BF16)
        craw_all = pB.tile([DH + 1, HPC, L], BF16)
        ctxT = pB.tile([P, CD // P, L], BF16)

        def rotb_at(j, t):
            return (rotb_q if j == 0 else rotb_k)[:, t, :, :]

        # ============ Phase 1: LN1 + QKV (our 768 cols) + partial stats + raw rope
        with ExitStack() as phA:
            pA = phA.enter_context(tc.tile_pool(name="pA", bufs=1))
            stats_pack = pA.tile([P, TT, 2, 2], F32)
            allred = pA.tile([P, TT, 2, 2], F32)
            rot_q = pA.tile([P, TT, HPC, DH], F32)
            rot_k = pA.tile([P, TT, HPC, DH], F32)

            def rot_at(j, t):
                return (rot_q if j == 0 else rot_k)[:, t, :, :]

            wq_sb = pA.tile([P, KC, CD], BF16)
            wk_sb = pA.tile([P, KC, CD], BF16)
            wv_sb = pA.tile([P, KC, CD], BF16)
            r1_sb = pA.tile([P, TT, DH], BF16)
            if use_lw:
                q4_all = pA.tile([P, TT, CD], F32)
                k4_all = pA.tile([P, TT, CD], F32)
                cos2_sb = pA.tile([P, TT, DH], BF16)
                nc.sync.dma_start(cos2_sb, cos_d)
                sinl2_sb = pA.tile([P, TT, DH // 2], BF16)
                nc.sync.dma_start(sinl2_sb, sinl_d)
                sinh2_sb = pA.tile([P, TT, DH // 2], BF16)
                nc.sync.dma_start(sinh2_sb, sinh_d)
                if use_qlw:
                    qlw_sb = pA.tile([P, CD], F32)
                    nc.sync.dma_start(qlw_sb, qlw_d.partition_broadcast(P)[:, 0, :])
                if use_klw:
                    klw_sb = pA.tile([P, CD], F32)
                    nc.sync.dma_start(klw_sb, klw_d.partition_broadcast(P)[:, 0, :])
            dramp = phA.enter_context(tc.tile_pool(name="dramp", bufs=1, space="DRAM"))
            ib1 = dramp.tile([P, TT * 2], F32)
            ob1 = dramp.tile([P, TT * 2], F32)
            ib2 = dramp.tile([P, TT * 2], F32)
            ob2 = dramp.tile([P, TT * 2], F32)

            def finalize_apply(lo, hi, after_group=None):
                """qk-LN: mu/rstd from all-reduced sums, fold into raw-rope'd q/k.
                rotb = rstd * (rot + (-mu) * r1): one DVE STT + one ACT scale per
                (tile, q/k). after_group(g4) fires once each 4-tile group is done."""
                n = hi - lo
                mu = small.tile([P, n, 2], F32, tag="fmu")
                nc.vector.tensor_scalar(mu, allred[:, lo:hi, :, 0], 1.0 / D, None, ALU.mult)
                m2 = small.tile([P, n, 2], F32, tag="fm2")
                nc.vector.tensor_mul(m2, mu, mu)
                rstd = small.tile([P, n, 2], F32, tag="frstd")
                nc.vector.scalar_tensor_tensor(
                    rstd, allred[:, lo:hi, :, 1], 1.0 / D, m2,
                    op0=ALU.mult, op1=ALU.subtract,
                )
                nc.scalar.activation(rstd, rstd, AF.Sqrt, bias=eps_ap)
                nc.vector.reciprocal(rstd, rstd)
                negmu = small.tile([P, n, 2], F32, tag="fnegmu")
                nc.vector.tensor_scalar(negmu, mu, -1.0, None, ALU.mult)
                nm = small.tile([P, n, 2], F32, tag="fnm")
                if use_lw:
                    nc.vector.scalar_tensor_tensor(nm, mu, -1.0, rstd, op0=ALU.mult, op1=ALU.mult)
                for t in range(lo, hi):
                    for j in range(2):
                        lw_flag = use_qlw if j == 0 else use_klw
                        if lw_flag:
                            src4 = q4_all if j == 0 else k4_all
                            lw_sb = qlw_sb if j == 0 else klw_sb
                            xn = small.tile([P, HPC, DH], F32, tag="xnf")
                            nc.scalar.activation(
                                xn.rearrange("p h d -> p (h d)"), src4[:, t, :],
                                AF.Identity, bias=nm[:, t - lo, j : j + 1],
                                scale=rstd[:, t - lo, j : j + 1],
                            )
                            nc.vector.tensor_mul(
                                xn, xn, lw_sb.rearrange("p (h d) -> p h d", h=HPC)
                            )
                            qa2 = small.tile([P, HPC, DH], F32, tag="qa2")
                            nc.vector.tensor_mul(
                                qa2, xn, _bcast_free(cos2_sb[:, t, :], HPC, 1)
                            )
                            qb2 = small.tile([P, HPC, DH], F32, tag="qb2")
                            nc.vector.tensor_mul(
                                qb2[:, :, 0 : DH // 2], xn[:, :, DH // 2 : DH],
                                _bcast_free(sinl2_sb[:, t, :], HPC, 1),
                            )
                            nc.vector.tensor_mul(
                                qb2[:, :, DH // 2 : DH], xn[:, :, 0 : DH // 2],
                                _bcast_free(sinh2_sb[:, t, :], HPC, 1),
                            )
                            nc.vector.tensor_add(rotb_at(j, t), qa2, qb2)
                            continue
                        tmp = small.tile([P, HPC, DH], F32, tag=f"tmp{j}", bufs=2)
                        nc.vector.scalar_tensor_tensor(
                            tmp, _bcast_free(r1_sb[:, t, :], HPC, 1),
                            negmu[:, t - lo, j : j + 1], rot_at(j, t),
                            op0=ALU.mult, op1=ALU.add,
                        )
                        nc.scalar.activation(
                            rotb_at(j, t).rearrange("p h d -> p (h d)"),
                            tmp.rearrange("p h d -> p (h d)"),
                            AF.Identity, scale=rstd[:, t - lo, j : j + 1],
                        )
                    if after_group is not None and t % 4 == 3:
                        after_group(t // 4)

            with ExitStack() as pctx:
                pp_qkv = pctx.enter_context(tc.tile_pool(name="pp_qkv", bufs=6, space="PSUM"))
                pp_ht = pctx.enter_context(tc.tile_pool(name="pp_ht", bufs=2, space="PSUM"))
                p1 = pctx.enter_context(tc.tile_pool(name="p1", bufs=2))

                x_pre = []
                for t in range(2):
                    x_t = p1.tile([P, D], BF16, tag="x_t", bufs=16, name=f"x_pre{t}")
                    nc.sync.dma_start(x_t, x_t_d[t])
                    x_pre.append(x_t)
                nc.sync.dma_start(wq_sb, wq_d)
                for t in range(2, 4):
                    x_t = p1.tile([P, D], BF16, tag="x_t", bufs=16, name=f"x_pre{t}")
                    nc.sync.dma_start(x_t, x_t_d[t])
                    x_pre.append(x_t)
                nc.sync.dma_start(wk_sb, wk_d)
                cos_sb = p1.tile([P, TT, DH], BF16, bufs=1)
                nc.sync.dma_start(cos_sb, cos_d)
                sinl_sb = p1.tile([P, TT, DH // 2], BF16, bufs=1)
                nc.sync.dma_start(sinl_sb, sinl_d)
                sinh_sb = p1.tile([P, TT, DH // 2], BF16, bufs=1)
                nc.sync.dma_start(sinh_sb, sinh_d)
                nc.sync.dma_start(wv_sb, wv_d)
                for t in range(4, 8):
                    x_t = p1.tile([P, D], BF16, tag="x_t", bufs=16, name=f"x_pre{t}")
                    nc.sync.dma_start(x_t, x_t_d[t])
                    x_pre.append(x_t)
                nc.sync.dma_start(r1_sb, r1_d)
                for t in range(8, TT):
                    x_t = p1.tile([P, D], BF16, tag="x_t", bufs=16, name=f"x_pre{t}")
                    nc.sync.dma_start(x_t, x_t_d[t])
                    x_pre.append(x_t)
                if use_ln1b:
                    lnb_sb = p1.tile([P, D], F32, bufs=1)
                    nc.sync.dma_start(lnb_sb, lnb_d.partition_broadcast(P)[:, 0, :])

                def stage1(t):
                    """LN1 + hT (DMA transpose) + QKV matmuls."""
                    x_t = x_pre[t]

                    xstats = small.tile([P, 2, 6], F32, tag="xstats")
                    for s in range(2):
                        nc.vector.bn_stats(
                            xstats[:, s, :],
                            x_t[:, s * 512 : (s + 1) * 512].rearrange(
                                "p (s d) -> p s d", s=1
                            ),
                        )
                    xmv = small.tile([P, 2], F32, tag="xmv")
                    nc.vector.bn_aggr(xmv, xstats)
                    xrstd = small.tile([P, 1], F32, tag="xrstd")
                    nc.scalar.activation(xrstd, xmv[:, 1:2], AF.Sqrt, bias=eps_ap)
                    nc.vector.reciprocal(xrstd, xrstd)
                    xnm = small.tile([P, 1], F32, tag="xnm")
                    nc.vector.tensor_scalar(xnm, xmv[:, 0:1], xrstd, -1.0, ALU.mult, ALU.mult)
                    h_t = p1.tile([P, D], BF16, tag="h_t", bufs=3)
                    nc.scalar.activation(h_t, x_t, AF.Identity, bias=xnm, scale=xrstd)
                    if use_ln1b:
                        nc.vector.tensor_add(h_t, h_t, lnb_sb)

                    ht_ps = pp_ht.tile([P, KC, P], BF16, tag="ht")
                    for c in range(KC):
                        nc.tensor.transpose(ht_ps[:, c, :], h_t[:, c * P : (c + 1) * P], identb)
                    hT_t = p1.tile([P, KC, P], BF16, tag="hT_t", bufs=2)
                    nc.scalar.copy(hT_t, ht_ps)

                    q_ps = pp_qkv.tile([P, CD], F32, tag="qkv", name="q_ps")
                    k_ps = pp_qkv.tile([P, CD], F32, tag="qkv", name="k_ps")
                    v_ps = pp_qkv.tile([P, CD], F32, tag="qkv", name="v_ps")
                    for c in range(KC):
                        nc.tensor.matmul(q_ps, hT_t[:, c, :], wq_sb[:, c, :],
                                         start=(c == 0), stop=(c == KC - 1))
                    for c in range(KC):
                        nc.tensor.matmul(k_ps, hT_t[:, c, :], wk_sb[:, c, :],
                                         start=(c == 0), stop=(c == KC - 1))
                    for c in range(KC):
                        nc.tensor.matmul(v_ps, hT_t[:, c, :], wv_sb[:, c, :],
                                         start=(c == 0), stop=(c == KC - 1))
                    return t, q_ps, k_ps, v_ps

                def stage2(st):
                    """Drain q/k/v psums, stats accums, raw rope."""
                    t, q_ps, k_ps, v_ps = st
                    if use_lw:
                        q4_t = q4_all[:, t, :]
                        k4_t = k4_all[:, t, :]
                    else:
                        q4_t = p1.tile([P, CD], F32, tag="q4t", bufs=3)
                        k4_t = p1.tile([P, CD], F32, tag="k4t", bufs=3)
                    nc.scalar.activation(
                        q4_t, q_ps, AF.Copy, accum_out=stats_pack[:, t, 0, 0:1]
                    )
                    nc.scalar.activation(
                        k4_t, k_ps, AF.Copy, accum_out=stats_pack[:, t, 1, 0:1]
                    )
                    nc.vector.tensor_copy(
                        v_blocks[:, t, :, 0:DH],
                        v_ps.rearrange("p (h d) -> p h d", h=HPC),
                    )
                    # s2 = rowsum(q4^2) on DVE, from SBUF
                    sq = p1.tile([P, CD], F32, tag="sq", bufs=2)
                    nc.vector.scalar_tensor_tensor(
                        sq, q4_t, 1.0, q4_t, op0=ALU.mult, op1=ALU.mult,
                        accum_out=stats_pack[:, t, 0, 1:2],
                    )
                    nc.vector.scalar_tensor_tensor(
                        sq, k4_t, 1.0, k4_t, op0=ALU.mult, op1=ALU.mult,
                        accum_out=stats_pack[:, t, 1, 1:2],
                    )

                    # raw rope (linear; LN affine folded in afterwards), from SBUF
                    for j in range(2):
                        src4 = q4_t if j == 0 else k4_t
                        xn4 = src4.rearrange("p (h d) -> p h d", h=HPC)
                        qa = p1.tile([P, HPC, DH], F32, tag="qa", bufs=2)
                        nc.vector.tensor_mul(qa, xn4, _bcast_free(cos_sb[:, t, :], HPC, 1))
                        qb = p1.tile([P, HPC, DH], F32, tag="qb", bufs=2)
                        nc.gpsimd.tensor_mul(
                            qb[:, :, 0 : DH // 2],
                            xn4[:, :, DH // 2 : DH],
                            _bcast_free(sinl_sb[:, t, :], HPC, 1),
                        )
                        nc.gpsimd.tensor_mul(
                            qb[:, :, DH // 2 : DH],
                            xn4[:, :, 0 : DH // 2],
                            _bcast_free(sinh_sb[:, t, :], HPC, 1),
                        )
                        nc.vector.tensor_add(rot_at(j, t), qa, qb)

                # two-stage software pipeline: stage1(t) runs one tile ahead of
                # stage2(t-1) so PE never waits on the psum-drain/rope tail.
                st_prev = None
                for t in range(TT):
                    st = stage1(t)
                    if st_prev is not None:
                        stage2(st_prev)
                        if t == 8:
                            # first-half AllReduce; hides under tiles 8-15
                            nc.gpsimd.dma_start(
                                ib1[:],
                                stats_pack[:, 0:8, :, :].rearrange("p t j s -> p (t j s)"),
                            )
                            nc.gpsimd.collective_compute(
                                "AllReduce", ALU.add, replica_groups=RG,
                                ins=[ib1.opt()], outs=[ob1.opt()],
                            )
                            nc.sync.dma_start(
                                allred[:, 0:8, :, :].rearrange("p t j s -> p (t j s)"),
                                ob1[:],
                            )
                    st_prev = st
                stage2(st_prev)

            # late constants: mask rows + fp32 identity (not needed until here)
            for hh in range(HPC):
                nc.sync.dma_start(qT[DH : DH + 5, hh, :], mq_d)
                nc.sync.dma_start(kT[DH : DH + 5, hh, :], mk_d)

            # second-half AllReduce launched immediately; its latency is hidden
            # under finalize(0:8) + first-half transposes + phase-A attention.
            nc.gpsimd.dma_start(
                ib2[:],
                stats_pack[:, 8:16, :, :].rearrange("p t j s -> p (t j s)"),
            )
            nc.gpsimd.collective_compute(
                "AllReduce", ALU.add, replica_groups=RG,
                ins=[ib2.opt()], outs=[ob2.opt()],
            )
            nc.sync.dma_start(
                allred[:, 8:16, :, :].rearrange("p t j s -> p (t j s)"),
                ob2[:],
            )

            # ============ Phase 2: attention (A while AR2 in flight, then B)
            with ExitStack() as actx:
                pa_sc = actx.enter_context(tc.tile_pool(name="pa_sc", bufs=3, space="PSUM"))
                pa_ctx = actx.enter_context(tc.tile_pool(name="pa_ctx", bufs=3, space="PSUM"))
                pp_tr = actx.enter_context(tc.tile_pool(name="pp_tr", bufs=2, space="PSUM"))
                p2 = actx.enter_context(tc.tile_pool(name="p2", bufs=4))

                def emit_transposes_group(g4):
                    for j in range(2):
                        dst = qT if j == 0 else kT
                        for hh in range(HPC):
                            tp = pp_tr.tile([DH, 4, P], BF16, tag="tp",
                                            name=f"tp{j}{hh}{g4}")
                            for i in range(4):
                                nc.tensor.transpose(
                                    tp[:, i, :], rotb_at(j, g4 * 4 + i)[:, hh, :],
                                    identb,
                                )
                            if hh % 2 == 0:
                                nc.scalar.copy(
                                    dst[0:DH, hh, g4 * 512 : (g4 + 1) * 512],
                                    tp.rearrange("p g t -> p (g t)"),
                                )
                            else:
                                nc.vector.tensor_copy(
                                    dst[0:DH, hh, g4 * 512 : (g4 + 1) * 512],
                                    tp.rearrange("p g t -> p (g t)"),
                                )

                def emit_attn(units):
                    # flatten to chunks of <=2 key tiles; 2-chunk software pipeline
                    chunks = []
                    for ui, (h, g, kts, mode) in enumerate(units):
                        for ci in range(0, len(kts), 2):
                            sub = kts[ci : ci + 2]
                            chunks.append({
                                "h": h, "g": g, "kts": sub, "mode": mode, "ui": ui,
                                "first": ci == 0, "last": ci + 2 >= len(kts),
                            })
                    cur_ctx = [None]

                    def emit_pv(c):
                        nk = len(c["kts"])
                        for i, kt in enumerate(c["kts"]):
                            voff = (kt * HPC + c["h"]) * VB
                            nc.tensor.matmul(
                                c["ctx"], v_sb[:, voff : voff + P],
                                c["eT"][:, i * QG : (i + 1) * QG],
                                start=(c["first"] and i == 0),
                                stop=(c["last"] and i == nk - 1),
                            )
                        if c["last"]:
                            dst = craw_all[:, c["h"], c["g"] * QG : (c["g"] + 1) * QG]
                            if c["mode"] == "add":
                                nc.vector.tensor_add(dst, dst, c["ctx"][0 : DH + 1, :])
                            else:
                                nc.vector.tensor_copy(dst, c["ctx"][0 : DH + 1, :])

                    pend = []
                    for c in chunks:
                        nk = len(c["kts"])
                        s_ps = pa_sc.tile([P, 2 * QG], F32, tag="sc", name="s_ps")
                        for i, kt in enumerate(c["kts"]):
                            nc.tensor.matmul(
                                s_ps[:, i * QG : (i + 1) * QG],
                                kT[0:KR, c["h"], kt * P : (kt + 1) * P],
                                qT[0:KR, c["h"], c["g"] * QG : (c["g"] + 1) * QG],
                                start=True, stop=True,
                            )
                        eT = p2.tile([P, 2 * QG], BF16, tag="eT", name="eT")
                        nc.scalar.activation(
                            eT[:, : nk * QG], s_ps[:, : nk * QG], AF.Exp
                        )
                        c["eT"] = eT
                        if c["first"]:
                            cur_ctx[0] = pa_ctx.tile([P, QG], F32, tag="ctx",
                                                     name="ctx_ps", bufs=3)
                        c["ctx"] = cur_ctx[0]
                        pend.append(c)
                        if len(pend) > 2:
                            emit_pv(pend.pop(0))
                    for c in pend:
                        emit_pv(c)

                wo_sb = p2.tile([P, CD // P, D], BF16, tag="wo", bufs=1)
                nc.sync.dma_start(wo_sb, wo_d)

                def emit_outproj_grp(grp):
                    """Normalize craw for qgroups 2*grp..2*grp+1 and project out."""
                    for h in range(HPC):
                        pr, sub = h // 2, h % 2
                        rows = slice(sub * DH, (sub + 1) * DH)
                        fwd = pp_tr.tile([P, 4, DH + 2], BF16, tag="tp",
                                         name=f"fwd{grp}{h}")
                        for i in range(4):
                            tix = grp * 4 + i
                            nc.tensor.transpose(
                                fwd[:, i, 0 : DH + 1],
                                craw_all[:, h, tix * P : (tix + 1) * P],
                                identb[: DH + 1, : DH + 1],
                            )
                        rz = small.tile([P, 4], F32, tag="rz")
                        nc.vector.reciprocal(rz, fwd[:, :, DH])
                        cn = p2.tile([P, 4, DH], BF16, tag="cn", bufs=2,
                                     name=f"cn{grp}{h}")
                        nc.vector.tensor_mul(cn, fwd[:, :, 0:DH], _bcast_free(rz, DH, 2))
                        back = pp_tr.tile([DH, 4, P], BF16, tag="tp",
                                          name=f"back{grp}{h}")
                        for i in range(4):
                            nc.tensor.transpose(back[:, i, :], cn[:, i, :], identb)
                        nc.vector.tensor_copy(
                            ctxT[rows, pr, grp * 512 : (grp + 1) * 512],
                            back.rearrange("p g t -> p (g t)"),
                        )
                    for i in range(4):
                        t = grp * 4 + i
                        o_ps = [pa_ctx.tile([P, 512], F32, tag="ctx", bufs=3,
                                            name=f"o_ps{t}{s}")
                                for s in range(2)]
                        for s in range(2):
                            for c in range(CD // P):
                                nc.tensor.matmul(
                                    o_ps[s], ctxT[:, c, t * P : (t + 1) * P],
                                    wo_sb[:, c, s * 512 : (s + 1) * 512],
                                    start=(c == 0), stop=(c == CD // P - 1),
                                )
                        o_t = p2.tile([P, D], BF16, tag="o_t", bufs=2, name=f"o_t{t}")
                        nc.scalar.copy(o_t[:, 0:512], o_ps[0])
                        nc.vector.tensor_copy(o_t[:, 512:1024], o_ps[1])
                        nc.sync.dma_start(out_t_d[t], o_t)

                finalize_apply(0, 8, after_group=emit_transposes_group)
                emit_attn(units_a)
                finalize_apply(8, 16, after_group=emit_transposes_group)
                # phase B grouped by qgroup pairs so each outproj group starts
                # as soon as its craw columns close
                ub = sorted(units_b, key=lambda u: (u[1], u[0]))
                for grp in range(4):
                    gs = (2 * grp, 2 * grp + 1)
                    emit_attn([u for u in ub if u[1] in gs])
                    emit_outproj_grp(grp)

    nc.compile()
    return nc


_CACHE = {}


def _get_nc(key):
    if key not in _CACHE:
        _CACHE[key] = build_bass(*key)
    return _CACHE[key]


def _qg_ranges(seq_id):
    """Per 256-query group: key-tile range (lo, hi) needed, unioned over batches.
    Falls back to dense if any row is unsorted."""
    for b in range(seq_id.shape[0]):
        if np.any(np.diff(seq_id[b].astype(np.int64)) < 0):
            return tuple((0, TT) for _ in range(NG))
    rs = []
    for g in range(NG):
        lo, hi = L, 0
        for b in range(seq_id.shape[0]):
            s = seq_id[b]
            l = int(np.searchsorted(s, s[g * QG], 'left'))
            h_ = int(np.searchsorted(s, s[(g + 1) * QG - 1], 'right'))
            lo = min(lo, l)
            hi = max(hi, h_)
        rs.append((lo // P, -(-hi // P)))
    return tuple(rs)


def _host_prep(x, seq_id, ln1_w, ln1_b, w_qkv, q_ln_w, k_ln_w, w_out):
    """Build the 8 per-core input maps."""
    x = np.asarray(x, np.float32)
    seq_id = np.asarray(seq_id)
    ln1_w = np.asarray(ln1_w, np.float32)
    ln1_b = np.asarray(ln1_b, np.float32)
    w_qkv = np.asarray(w_qkv, np.float32)
    q_ln_w = np.asarray(q_ln_w, np.float32)
    k_ln_w = np.asarray(k_ln_w, np.float32)
    w_out = np.asarray(w_out, np.float32)

    use_ln1b = bool(np.any(ln1_b != 0.0))
    use_qlw = not np.allclose(q_ln_w, 1.0)
    use_klw = not np.allclose(k_ln_w, 1.0)
    qgr = _qg_ranges(seq_id)

    BD = ml_dtypes.bfloat16
    wq_f = (w_qkv[:, 0:D] * ln1_w[:, None]).astype(BD)
    wk_f = (w_qkv[:, D : 2 * D] * ln1_w[:, None]).astype(BD)
    wv_f = (w_qkv[:, 2 * D : 3 * D] * ln1_w[:, None]).astype(BD)
    wo_b = w_out.astype(BD)
    x_b = x.astype(BD)

    # rope tables, with 1/sqrt(sqrt(64)) on each side -> scores * 1/8
    inv_freq = 1.0 / (ROPE_BASE ** (np.arange(0, DH, 2, dtype=np.float32) / DH))
    tpos = np.arange(L, dtype=np.float32)
    freqs = np.einsum("l,f->lf", tpos, inv_freq)
    emb = np.concatenate([freqs, freqs], axis=-1)
    s8 = np.float32(8.0 ** -0.5)
    cos_t = (np.cos(emb) * s8).astype(np.float32)
    sin_t = (np.sin(emb) * s8).astype(np.float32)
    sinl = -sin_t[:, : DH // 2]
    sinh = sin_t[:, DH // 2 :]
    r1 = np.concatenate(
        [cos_t[:, : DH // 2] - sin_t[:, : DH // 2],
         cos_t[:, DH // 2 :] + sin_t[:, DH // 2 :]], axis=1
    )

    identb = np.eye(P, dtype=BD)

    def wlay(a):   # [D, n] -> [P, KC, n] with d = c*P + p
        return np.ascontiguousarray(a.reshape(KC, P, -1).transpose(1, 0, 2))

    def tlay(a):   # [L, d] -> [P, TT, d] with l = n*P + p
        return np.ascontiguousarray(a.reshape(TT, P, -1).transpose(1, 0, 2))

    in_maps = []
    for c in range(8):
        b, g = c // HPC, c % HPC
        mine = np.arange(g * CD, (g + 1) * CD)

        sid = np.asarray(seq_id[b], np.int64)
        A = (sid[None, :] == np.arange(4)[:, None]).astype(np.float32)
        maskq = np.concatenate([MASK_A * A, MASK_A * np.ones((1, L), np.float32)])
        maskk = np.concatenate([MASK_A * A, -MASK_A * np.ones((1, L), np.float32)])

        m = {
            "x": np.ascontiguousarray(x_b[b]),
            "wq": wlay(wq_f[:, mine]),
            "wk": wlay(wk_f[:, mine]),
            "wv": wlay(wv_f[:, mine]),
            "wo": np.ascontiguousarray(
                wo_b[mine, :].reshape(CD // P, P, D).transpose(1, 0, 2)),
            "maskq": maskq.astype(BD),
            "maskk": maskk.astype(BD),
            "cos": tlay(cos_t.astype(BD)),
            "sinl": tlay(np.ascontiguousarray(sinl).astype(BD)),
            "sinh": tlay(np.ascontiguousarray(sinh).astype(BD)),
            "r1": tlay(np.ascontiguousarray(r1).astype(BD)),
            "identb": identb,
        }
        if use_ln1b:
            m["lnb"] = ln1_b.reshape(1, D)
        if use_qlw:
            m["qlw"] = q_ln_w[mine].reshape(1, CD)
        if use_klw:
            m["klw"] = k_ln_w[mine].reshape(1, CD)
        in_maps.append(m)
    return in_maps, (use_ln1b, use_qlw, use_klw, qgr)


def run(inputs, trace=False):
    """Run on hardware; returns (output [B, L, D] fp32, BassKernelResults)."""
    in_maps, key = _host_prep(**inputs)
    nc = _get_nc(key)
    res = bass_utils.run_bass_kernel_spmd(
        nc, in_maps, core_ids=list(range(8)), trace=trace
    )
    out = np.zeros((B, L, D), np.float32)
    for c in range(8):
        out[c // HPC] += np.asarray(res.results[c]["out"], dtype=np.float32)
    return out, res


def kernel(**inputs) -> np.ndarray:
    out, _ = run(inputs)
    return out
